# revision 1
# baseline (speedup 1.0000x reference)
"""Trainium2 Bass kernel for nn_DAGNLinkPrediction (GNN message passing).

Self-contained: host-side integer preprocessing (sharding/permutation) + bass/tile
kernel + SPMD launch across 8 NeuronCores via run_bass_kernel_spmd.

Sharding: edges partitioned by src range (6272 nodes/core). Per core, edges are
sorted by (dst>=SPLIT, src-block, src) and padded into 128-edge chunks confined
to 128-node blocks. Per power iteration each core gathers Z[dst] rows (256B bf16)
with gpsimd.dma_gather, computes messages, segment-sums by src via PE matmuls
with 0/1 one-hot matrices, and AllGathers the updated bf16 node table.
"""
import numpy as np
import ml_dtypes

from concourse import bass, bacc, tile, bass_utils, mybir

BF = ml_dtypes.bfloat16
F32 = mybir.dt.float32
BF16 = mybir.dt.bfloat16
I16 = mybir.dt.int16

CORES = 8
N_ENT = 50000
N_REL = 500
HEADS = 2
DIM = 64
HD = HEADS * DIM                 # 128
M_PER_CORE = 6272                # 49*128
NBLK = M_PER_CORE // 128         # 49
TAB_ROWS = CORES * M_PER_CORE    # 50176
SPLIT = TAB_ROWS // 2            # 25088
NPIECE = 7                       # pieces per pass; NBLK = 7*7
BPP = NBLK // NPIECE             # blocks per piece = 7
ALPHA = 0.15
LN_EPS = 1e-5
L_LAYERS = 2
POW_ITER = 3
LRELU = 0.01

AF = mybir.ActivationFunctionType
OP = mybir.AluOpType


# ----------------------------------------------------------------------------
# host-side preprocessing (integer/layout only)
# ----------------------------------------------------------------------------

def _wrap_idxs(idx):
    n = idx.shape[0]
    base = idx.reshape(n // 16, 16).T.astype(np.int16)
    return np.ascontiguousarray(np.tile(base, (8, 1)))


def _build_plan(edge_index, edge_type):
    src = edge_index[0].astype(np.int64)
    dst = edge_index[1].astype(np.int64)
    et = edge_type.astype(np.int64)

    core_of = src // M_PER_CORE
    hi = (dst >= SPLIT).astype(np.int64)
    blk = (src % M_PER_CORE) // 128
    key = (core_of * 2 + hi) * NBLK + blk
    order = np.argsort(key, kind="stable")
    key_sorted = key[order]
    bounds = np.searchsorted(key_sorted, np.arange(CORES * 2 * NBLK + 1))

    counts = (bounds[1:] - bounds[:-1]).reshape(CORES, 2, NBLK)
    CPB = max(1, int(np.ceil(counts.max() / 128)))
    C_PASS = NBLK * CPB
    NIDX = C_PASS * 128

    plans = []
    for c in range(CORES):
        pl = {}
        pl["node_lo"] = c * M_PER_CORE
        pl["n_valid"] = max(0, min(N_ENT - c * M_PER_CORE, M_PER_CORE))
        gidx, offs, typs = [], [], []
        for p in range(2):
            eids = np.full(NIDX, -1, np.int64)
            for b in range(NBLK):
                k = (c * 2 + p) * NBLK + b
                lst = order[bounds[k]:bounds[k + 1]]
                eids[b * CPB * 128: b * CPB * 128 + len(lst)] = lst
            pad = eids < 0
            e_safe = np.where(pad, 0, eids)
            didx = dst[e_safe] - p * SPLIT
            didx[pad] = 0
            off = (src[e_safe] % 128).astype(np.int64)
            off[pad] = -1
            t = et[e_safe].copy()
            t[pad] = 0
            gidx.append(_wrap_idxs(didx.astype(np.int16)))
            offs.append(off.reshape(C_PASS, 128).T.astype(np.float32))  # [128, C_PASS]
            typs.append(t)
        pl["gidx"] = gidx
        pl["tidx"] = _wrap_idxs(np.concatenate(typs).astype(np.int16))
        shl = []
        for p in range(2):
            off_flat = offs[p].T.reshape(-1)  # [NIDX] slot-order src offsets (-1 pads)
            # rebuild local src ids from block index + offset
            blk_of_slot = np.repeat(np.arange(NBLK), CPB * 128)
            sl = blk_of_slot * 128 + np.maximum(off_flat, 0).astype(np.int64)
            sl[off_flat < 0] = 0
            shl.append(sl)
        pl["shidx"] = _wrap_idxs(np.concatenate(shl).astype(np.int16))
        pl["offs"] = np.concatenate(offs, axis=1).astype(BF)  # [128, 2*C_PASS]
        plans.append(pl)

    meta = dict(CPB=CPB, C_PASS=C_PASS, NIDX=NIDX,
                PIECE_CHUNKS=BPP * CPB, NIDX_PIECE=BPP * CPB * 128)
    return plans, meta


# ----------------------------------------------------------------------------
# device kernel
# ----------------------------------------------------------------------------

def _build_nc(meta, debug=False, n_layers=L_LAYERS, n_iters=POW_ITER, do_ag=True, do_node_ag=True):
    CPB = meta["CPB"]
    C_PASS = meta["C_PASS"]
    PC = meta["PIECE_CHUNKS"]          # chunks per piece
    NP_IDX = meta["NIDX_PIECE"]        # idxs per piece
    IW = meta["NIDX"] // 16            # idx cols per pass

    nc = bacc.Bacc("TRN2", target_bir_lowering=False, debug=False,
                   num_devices=CORES)

    # ---- external inputs ----
    ent_in = nc.dram_tensor("ent_slice", [M_PER_CORE, DIM], F32, kind="ExternalInput")
    rel_in = nc.dram_tensor("rel_emb", [512, DIM], F32, kind="ExternalInput")
    lng_in = nc.dram_tensor("ln_g", [128, L_LAYERS, DIM], F32, kind="ExternalInput")
    lnb_in = nc.dram_tensor("ln_b", [128, L_LAYERS, DIM], F32, kind="ExternalInput")
    W_in = nc.dram_tensor("W_htr", [3, L_LAYERS, DIM, HD], F32, kind="ExternalInput")
    att_in = nc.dram_tensor("att_rep", [3, L_LAYERS, 128, HD], F32, kind="ExternalInput")
    Wo_in = nc.dram_tensor("W_o", [L_LAYERS, HD, DIM], F32, kind="ExternalInput")
    gidx_in = nc.dram_tensor("gidx", [128, 2, IW], I16, kind="ExternalInput")
    tidx_in = nc.dram_tensor("tidx", [128, 2 * IW], I16, kind="ExternalInput")
    shidx_in = nc.dram_tensor("shidx", [128, 2 * IW], I16, kind="ExternalInput")
    offs_in = nc.dram_tensor("offs", [128, 2 * C_PASS], BF16, kind="ExternalInput")
    iota_in = nc.dram_tensor("iota", [128, 128], BF16, kind="ExternalInput")
    idf_in = nc.dram_tensor("ident_f", [128, 128], F32, kind="ExternalInput")

    out_ext = nc.dram_tensor("out_slice", [M_PER_CORE, DIM], F32, kind="ExternalOutput")
    if debug:
        dbg_at = nc.dram_tensor("dbg_at", [128, 2 * C_PASS, 2], BF16, kind="ExternalOutput")
        dbg_z0 = nc.dram_tensor("dbg_z0", [M_PER_CORE, HD], F32, kind="ExternalOutput")
        dbg_h = nc.dram_tensor("dbg_h", [M_PER_CORE, DIM], F32, kind="ExternalOutput")

    with tile.TileContext(nc) as tc:
        with tc.tile_pool(name="dram", bufs=1, space="DRAM") as dram, \
             tc.tile_pool(name="persist", bufs=1) as pp:
            table = dram.tile([TAB_ROWS, 128], BF16, tag="table")
            tab_in = dram.tile([M_PER_CORE, 128], BF16, tag="tab_in")
            srtab = dram.tile([512, 128], BF16, tag="srtab")
            shtab = dram.tile([M_PER_CORE, 128], BF16, tag="shtab")
            ohcache = dram.tile([128, 2 * C_PASS, 128], BF16, tag="ohcache")

            ent = pp.tile([128, NBLK, DIM], F32, tag="ent")
            h_t = pp.tile([128, NBLK, DIM], F32, tag="h")
            recip = pp.tile([128, NBLK, 2], F32, tag="recip")
            zt = pp.tile([128, NBLK, 2], F32, tag="zt")
            At = pp.tile([128, 2 * C_PASS, 2], BF16, tag="At")
            SRx = pp.tile([128, 2 * C_PASS, 8], BF16, tag="SRx")
            Z = pp.tile([128, NBLK, HD], F32, tag="Z")
            gidx = pp.tile([128, 2, IW], I16, tag="gidx")
            tidx = pp.tile([128, 2 * IW], I16, tag="tidx")
            shidx = pp.tile([128, 2 * IW], I16, tag="shidx")
            offs = pp.tile([128, 2 * C_PASS], BF16, tag="offs")
            iota = pp.tile([128, 128], BF16, tag="iota")
            idf = pp.tile([128, 128], F32, tag="idf")
            lng = pp.tile([128, L_LAYERS, DIM], F32, tag="lng")
            lnb = pp.tile([128, L_LAYERS, DIM], F32, tag="lnb")
            Wht = pp.tile([64, 3 * L_LAYERS, HD], F32, tag="Wht")
            attr = pp.tile([128, 3 * L_LAYERS, HD], F32, tag="attr")
            Wo = pp.tile([HD, L_LAYERS, DIM], F32, tag="Wo")

            # ---- load inputs ----
            nc.sync.dma_start(ent[:, :, :], ent_in.ap().rearrange("(b p) f -> p b f", p=128))
            nc.sync.dma_start(gidx[:, :, :], gidx_in.ap())
            nc.sync.dma_start(tidx[:, :], tidx_in.ap())
            nc.sync.dma_start(shidx[:, :], shidx_in.ap())
            nc.sync.dma_start(offs[:, :], offs_in.ap())
            nc.sync.dma_start(iota[:, :], iota_in.ap())
            nc.sync.dma_start(idf[:, :], idf_in.ap())
            nc.sync.dma_start(lng[:, :, :], lng_in.ap())
            nc.sync.dma_start(lnb[:, :, :], lnb_in.ap())
            nc.sync.dma_start(Wht[:, :, :], W_in.ap().rearrange("r l k m -> k (r l) m"))
            nc.sync.dma_start(attr[:, :, :], att_in.ap().rearrange("r l p m -> p (r l) m"))
            nc.sync.dma_start(Wo[:, :, :], Wo_in.ap().rearrange("l k m -> k l m"))

            def node_scores_block(pool, psum, lhsT, Wslice, att_ap, out_ap):
                """tanh(x@W) . att summed over d -> out_ap [128,2] (f32)."""
                pt = psum.tile([128, HD], F32, tag="ns_ps")
                nc.tensor.matmul(pt[:, :], lhsT, Wslice, start=True, stop=True)
                tt = pool.tile([128, HD], F32, tag="ns_tt")
                nc.scalar.activation(tt[:, :], pt[:, :], AF.Tanh)
                tm = pool.tile([128, HD], F32, tag="ns_tm")
                nc.vector.tensor_tensor(tm[:, :], tt[:, :], att_ap, OP.mult)
                nc.vector.tensor_reduce(out_ap, tm.rearrange("p (h d) -> p h d", h=2),
                                        mybir.AxisListType.X, OP.add)

            for l in range(n_layers):
                # ================= node phase =================
                with tc.tile_pool(name="nodep", bufs=2) as np_pool, \
                     tc.tile_pool(name="nodebig", bufs=1) as np_big, \
                     tc.tile_pool(name="nodeps", bufs=2, space="PSUM") as np_psum:
                    # layernorm
                    x = ent
                    mu = np_pool.tile([128, NBLK], F32, tag="mu")
                    nc.vector.tensor_reduce(mu[:, :], x[:, :, :], mybir.AxisListType.X, OP.add)
                    nc.vector.tensor_scalar(mu[:, :], mu[:, :], 1.0 / DIM, None, OP.mult)
                    xc = np_big.tile([128, NBLK, DIM], F32, tag="xc")
                    nc.vector.tensor_tensor(
                        xc[:, :, :], x[:, :, :],
                        mu.unsqueeze(2).broadcast_to([128, NBLK, DIM]), OP.subtract)
                    sq = np_big.tile([128, NBLK, DIM], F32, tag="sq")
                    nc.vector.tensor_tensor(sq[:, :, :], xc[:, :, :], xc[:, :, :], OP.mult)
                    var = np_pool.tile([128, NBLK], F32, tag="var")
                    nc.vector.tensor_reduce(var[:, :], sq[:, :, :], mybir.AxisListType.X, OP.add)
                    nc.vector.tensor_scalar(var[:, :], var[:, :], 1.0 / DIM, LN_EPS, OP.mult, OP.add)
                    std = np_pool.tile([128, NBLK], F32, tag="std")
                    nc.scalar.activation(std[:, :], var[:, :], AF.Sqrt)
                    rstd = np_pool.tile([128, NBLK], F32, tag="rstd")
                    nc.vector.reciprocal(rstd[:, :], std[:, :])
                    nc.vector.tensor_tensor(
                        h_t[:, :, :], xc[:, :, :],
                        rstd.unsqueeze(2).broadcast_to([128, NBLK, DIM]), OP.mult)
                    nc.vector.tensor_tensor(
                        h_t[:, :, :], h_t[:, :, :],
                        lng[:, l, :].unsqueeze(1).broadcast_to([128, NBLK, DIM]),
                        OP.mult)
                    nc.vector.tensor_tensor(
                        h_t[:, :, :], h_t[:, :, :],
                        lnb[:, l, :].unsqueeze(1).broadcast_to([128, NBLK, DIM]),
                        OP.add)
                    if debug and l == 0:
                        nc.sync.dma_start(dbg_h.ap().rearrange("(b p) f -> p b f", p=128),
                                          h_t[:, :, :])

                    # transpose h -> ht [64, b, 128]
                    ht = np_big.tile([64, NBLK, 128], F32, tag="ht")
                    for b in range(NBLK):
                        ps = np_psum.tile([64, 128], F32, tag="trh")
                        nc.tensor.transpose(ps[:, :], h_t[:, b, :], idf[:, :])
                        nc.scalar.activation(ht[:, b, :], ps[:, :], AF.Copy)

                    # s_h, s_t  [128, NBLK, 2] f32
                    s_f0 = np_pool.tile([128, NBLK, 2], F32, tag="s_f0")
                    s_f1 = np_pool.tile([128, NBLK, 2], F32, tag="s_f1")
                    s_f = [s_f0, s_f1]
                    for r in range(2):
                        for b in range(NBLK):
                            node_scores_block(np_pool, np_psum, ht[:, b, :],
                                              Wht[:, r * L_LAYERS + l, :],
                                              attr[:, r * L_LAYERS + l, :],
                                              s_f[r][:, b, :])
                    # sh table rows: [bf16(s_h)(2), bf16(residual)(2), 0...]
                    shsb = np_big.tile([128, NBLK, 128], BF16, tag="shsb")
                    shm_f = np_pool.tile([128, NBLK, 2], F32, tag="shm_f")
                    nc.vector.memset(shsb[:, :, 4:128], 0.0)
                    nc.vector.tensor_copy(shsb[:, :, 0:2], s_f[0][:, :, :])
                    nc.vector.tensor_copy(shm_f[:, :, :], shsb[:, :, 0:2])
                    nc.vector.tensor_tensor(shsb[:, :, 2:4], s_f[0][:, :, :], shm_f[:, :, :],
                                            OP.subtract)
                    nc.sync.dma_start(shtab.rearrange("(b p) c -> p b c", p=128),
                                      shsb[:, :, :])

                    # gather table slice: [h | s_t | s_t_res | 1 | 0...]
                    tabsb = np_big.tile([128, NBLK, 128], BF16, tag="tabsb")
                    nc.vector.tensor_copy(tabsb[:, :, 0:64], h_t[:, :, :])
                    nc.vector.tensor_copy(tabsb[:, :, 64:66], s_f[1][:, :, :])
                    stm_f = np_pool.tile([128, NBLK, 2], F32, tag="stm_f")
                    nc.vector.tensor_copy(stm_f[:, :, :], tabsb[:, :, 64:66])
                    nc.vector.tensor_tensor(tabsb[:, :, 66:68], s_f[1][:, :, :], stm_f[:, :, :],
                                            OP.subtract)
                    nc.vector.memset(tabsb[:, :, 68:69], 1.0)
                    nc.vector.memset(tabsb[:, :, 69:128], 0.0)
                    nc.sync.dma_start(tab_in.rearrange("(b p) c -> p b c", p=128),
                                      tabsb[:, :, :])
                    if do_node_ag:
                        nc.gpsimd.collective_compute(
                            "AllGather", OP.bypass,
                            replica_groups=[list(range(CORES))],
                            ins=[tab_in.opt()], outs=[table.opt()])
                    else:
                        nc.sync.dma_start(table[0:M_PER_CORE, :],
                                          tab_in[:, :])

                    # s_r table (once, both layers)
                    if l == 0:
                        relsb = np_pool.tile([128, 4, DIM], F32, tag="relsb")
                        nc.sync.dma_start(relsb[:, :, :],
                                          rel_in.ap().rearrange("(b p) f -> p b f", p=128))
                        relt = np_pool.tile([64, 4, 128], F32, tag="relt")
                        for b in range(4):
                            ps = np_psum.tile([64, 128], F32, tag="trh")
                            nc.tensor.transpose(ps[:, :], relsb[:, b, :], idf[:, :])
                            nc.scalar.activation(relt[:, b, :], ps[:, :], AF.Copy)
                        srsb = np_pool.tile([128, 4, 128], BF16, tag="srsb")
                        nc.vector.memset(srsb[:, :, :], 0.0)
                        sr_f = np_pool.tile([128, 4, 2 * L_LAYERS], F32, tag="sr_f")
                        for ll in range(L_LAYERS):
                            for b in range(4):
                                node_scores_block(np_pool, np_psum, relt[:, b, :],
                                                  Wht[:, 2 * L_LAYERS + ll, :],
                                                  attr[:, 2 * L_LAYERS + ll, :],
                                                  sr_f[:, b, 2 * ll:2 * ll + 2])
                        nc.vector.tensor_copy(srsb[:, :, 0:4], sr_f[:, :, :])
                        srm_f = np_pool.tile([128, 4, 4], F32, tag="srm_f")
                        nc.vector.tensor_copy(srm_f[:, :, :], srsb[:, :, 0:4])
                        nc.vector.tensor_tensor(srsb[:, :, 4:8], sr_f[:, :, :], srm_f[:, :, :],
                                                OP.subtract)
                        nc.sync.dma_start(srtab.rearrange("(b p) c -> p b c", p=128),
                                          srsb[:, :, :])

                # ================= edge phase =================
                for it in range(n_iters):
                    with tc.tile_pool(name="edgep", bufs=3) as ep, \
                         tc.tile_pool(name="edgeps", bufs=2) as eps, \
                         tc.tile_pool(name="edgep2", bufs=2) as ep2, \
                         tc.tile_pool(name="spmm_ps", bufs=2, space="PSUM") as spmm_ps, \
                         tc.tile_pool(name="zps_pool", bufs=2, space="PSUM") as zps_pool:
                        for k in range(NPIECE):
                            psb = spmm_ps.tile([128, BPP, 128], F32, tag="blkps")
                            psz = zps_pool.tile([128, BPP, 2], F32, tag="zps")
                            for p in range(2):
                                slot0 = p * C_PASS + k * PC
                                Gt = ep.tile([128, PC, 128], BF16, tag="Gt")
                                nc.gpsimd.dma_gather(
                                    out_ap=Gt[:, :, :],
                                    in_ap=table[p * SPLIT:, :],
                                    idxs_ap=gidx[:, p, k * (NP_IDX // 16):(k + 1) * (NP_IDX // 16)],
                                    num_idxs=NP_IDX, num_idxs_reg=NP_IDX, elem_size=128, single_packet=False)
                                # plain one-hot [128e, c, 128n]; built on DVE in
                                # iters 0/1, spilled to DRAM in iter 1, re-read in
                                # iter 2 (identical content across iterations).
                                oh = ep2.tile([128, PC, 128], BF16, tag="oh")
                                cache_ready = n_iters > 2 and (l > 0 or it > 1)
                                if cache_ready and not (it == 0):
                                    nc.sync.dma_start(oh[:, :, :],
                                                      ohcache[:, slot0:slot0 + PC, :])
                                else:
                                    nc.vector.tensor_tensor(
                                        oh[:, :, :],
                                        offs[:, slot0:slot0 + PC].unsqueeze(2)
                                            .broadcast_to([128, PC, 128]),
                                        iota.unsqueeze(1).broadcast_to([128, PC, 128]),
                                        OP.is_equal)
                                    if l == 0 and it == 1 and n_iters > 2:
                                        nc.sync.dma_start(ohcache[:, slot0:slot0 + PC, :],
                                                          oh[:, :, :])

                                if it == 0:
                                    idx0 = (p * NPIECE + k) * (NP_IDX // 16)
                                    idx1 = (p * NPIECE + k + 1) * (NP_IDX // 16)
                                    if l == 0:
                                        SRt = eps.tile([128, PC, 128], BF16, tag="SRt")
                                        nc.gpsimd.dma_gather(
                                            out_ap=SRt[:, :, :],
                                            in_ap=srtab[:, :],
                                            idxs_ap=tidx[:, idx0:idx1],
                                            num_idxs=NP_IDX, num_idxs_reg=NP_IDX, elem_size=128, single_packet=False)
                                        nc.vector.tensor_copy(
                                            SRx[:, slot0:slot0 + PC, :], SRt[:, :, 0:8])
                                    SHt = eps.tile([128, PC, 128], BF16, tag="SHt")
                                    nc.gpsimd.dma_gather(
                                        out_ap=SHt[:, :, :],
                                        in_ap=shtab[:, :],
                                        idxs_ap=shidx[:, idx0:idx1],
                                        num_idxs=NP_IDX, num_idxs_reg=NP_IDX, elem_size=128, single_packet=False)
                                    # scores
                                    sc = ep2.tile([128, PC, 2], F32, tag="sc")
                                    nc.vector.tensor_tensor(sc[:, :, :], Gt[:, :, 64:66],
                                                            Gt[:, :, 66:68], OP.add)
                                    t2 = ep2.tile([128, PC, 2], F32, tag="t2")
                                    nc.vector.tensor_tensor(
                                        t2[:, :, :],
                                        SRx[:, slot0:slot0 + PC, 2 * l:2 * l + 2],
                                        SRx[:, slot0:slot0 + PC, 4 + 2 * l:6 + 2 * l], OP.add)
                                    nc.vector.tensor_tensor(sc[:, :, :], sc[:, :, :], t2[:, :, :],
                                                            OP.add)
                                    nc.vector.tensor_tensor(t2[:, :, :], SHt[:, :, 0:2],
                                                            SHt[:, :, 2:4], OP.add)
                                    nc.vector.tensor_tensor(sc[:, :, :], sc[:, :, :], t2[:, :, :],
                                                            OP.add)
                                    nc.vector.scalar_tensor_tensor(
                                        sc[:, :, :], sc[:, :, :], LRELU, sc[:, :, :],
                                        OP.mult, OP.max)
                                    nc.scalar.activation(At[:, slot0:slot0 + PC, :],
                                                         sc[:, :, :], AF.Exp)

                                # messages (in place into Gt; head1 first)
                                g1 = 0 if it == 0 else 64
                                msg = Gt
                                At0 = At[:, slot0:slot0 + PC, 0].unsqueeze(2) \
                                    .broadcast_to([128, PC, 64])
                                At1 = At[:, slot0:slot0 + PC, 1].unsqueeze(2) \
                                    .broadcast_to([128, PC, 64])
                                nc.vector.tensor_tensor(msg[:, :, 64:128],
                                                        Gt[:, :, g1:g1 + 64], At1, OP.mult)
                                nc.vector.tensor_tensor(msg[:, :, 0:64], Gt[:, :, 0:64], At0,
                                                        OP.mult)
                                # spmm (+ z columns via separate 2-col matmul on iter 0)
                                # PSUM zero-regions are whole banks: start/stop once per bank.
                                bank1_j0 = 4 * CPB
                                for j in range(PC):
                                    bl = j // CPB
                                    st = (p == 0) and (j == 0 or j == bank1_j0)
                                    sp = (p == 1) and (j == bank1_j0 - 1 or j == PC - 1)
                                    nc.tensor.matmul(
                                        psb[:, bl, :], oh[:, j, :], msg[:, j, :],
                                        start=st, stop=sp)
                                    if it == 0:
                                        nc.tensor.matmul(
                                            psz[:, bl, :], oh[:, j, :],
                                            At[:, slot0 + j, :],
                                            start=(p == 0 and j == 0),
                                            stop=(p == 1 and j == PC - 1))

                            # piece epilogue: z, recip, Z assembly
                            if it == 0:
                                b0 = k * BPP
                                nc.vector.tensor_scalar(zt[:, b0:b0 + BPP, :], psz[:, :, :],
                                                        1e-30, None, OP.max)
                                nc.vector.reciprocal(recip[:, b0:b0 + BPP, :],
                                                     zt[:, b0:b0 + BPP, :])
                                nc.vector.tensor_scalar(recip[:, b0:b0 + BPP, :],
                                                        recip[:, b0:b0 + BPP, :],
                                                        1.0 - ALPHA, None, OP.mult)
                            for bl in range(BPP):
                                b = k * BPP + bl
                                nc.scalar.activation(Z[:, b, 0:64], psb[:, bl, 0:64], AF.Copy,
                                                     scale=recip[:, b, 0:1])
                                nc.scalar.activation(Z[:, b, 64:128], psb[:, bl, 64:128],
                                                     AF.Copy, scale=recip[:, b, 1:2])
                                nc.vector.scalar_tensor_tensor(
                                    Z[:, b, :].rearrange("p (o d) -> p o d", o=2),
                                    h_t[:, b, :].unsqueeze(1)
                                        .broadcast_to([128, 2, 64]),
                                    ALPHA,
                                    Z[:, b, :].rearrange("p (o d) -> p o d", o=2),
                                    OP.mult, OP.add)

                        if debug and l == 0 and it == 0:
                            nc.sync.dma_start(dbg_z0.ap().rearrange("(b p) c -> p b c", p=128),
                                              Z[:, :, :])
                            nc.sync.dma_start(
                                dbg_at.ap(),
                                At[:, :, :])

                        if it < n_iters - 1:
                            tabz = ep.tile([128, NBLK, 128], BF16, tag="Gt")
                            nc.vector.tensor_copy(tabz[:, :, :], Z[:, :, :])
                            nc.sync.dma_start(tab_in.rearrange("(b p) c -> p b c", p=128),
                                              tabz[:, :, :])
                            if do_ag:
                                nc.gpsimd.collective_compute(
                                    "AllGather", OP.bypass,
                                    replica_groups=[list(range(CORES))],
                                    ins=[tab_in.opt()], outs=[table.opt()])

                # ================= conv + residual =================
                with tc.tile_pool(name="convp", bufs=2) as cp, \
                     tc.tile_pool(name="convps", bufs=4, space="PSUM") as cps:
                    for b in range(NBLK):
                        pzt = cps.tile([128, 128], F32, tag="pzt")
                        nc.tensor.transpose(pzt[:, :], Z[:, b, :], idf[:, :])
                        Zt = cp.tile([128, 128], F32, tag="Zt")
                        nc.scalar.activation(Zt[:, :], pzt[:, :], AF.Copy)
                        pc_ = cps.tile([128, 64], F32, tag="pc")
                        nc.tensor.matmul(pc_[:, :], Zt[:, :], Wo[:, l, :],
                                         start=True, stop=True)
                        nc.vector.tensor_tensor(ent[:, b, :], pc_[:, :], ent[:, b, :], OP.add)

            nc.sync.dma_start(out_ext.ap().rearrange("(b p) f -> p b f", p=128),
                              ent[:, :, :])

    nc.compile()
    return nc


# ----------------------------------------------------------------------------
# host wrapper
# ----------------------------------------------------------------------------

def _make_in_maps(inputs, plans):
    ent = np.asarray(inputs["entity_embed"], np.float32)
    rel = np.zeros((512, DIM), np.float32)
    rel[:N_REL] = np.asarray(inputs["relation_embed"], np.float32)
    lng = np.tile(np.asarray(inputs["ln_gamma"], np.float32)[None], (128, 1, 1))
    lnb = np.tile(np.asarray(inputs["ln_beta"], np.float32)[None], (128, 1, 1))
    W = np.stack([np.asarray(inputs["W_h"], np.float32),
                  np.asarray(inputs["W_t"], np.float32),
                  np.asarray(inputs["W_r"], np.float32)], axis=0)
    att = np.stack([np.asarray(inputs["att_h"], np.float32),
                    np.asarray(inputs["att_t"], np.float32),
                    np.asarray(inputs["att_r"], np.float32)], axis=0)
    att_rep = np.tile(att.reshape(3, L_LAYERS, 1, HD), (1, 1, 128, 1)).astype(np.float32)
    Wo = np.asarray(inputs["W_o"], np.float32)
    iota = np.tile(np.arange(128, dtype=np.float32)[None], (128, 1)).astype(BF)
    idf = np.eye(128, dtype=np.float32)

    common = dict(rel_emb=rel, ln_g=np.ascontiguousarray(lng), ln_b=np.ascontiguousarray(lnb),
                  W_htr=np.ascontiguousarray(W), att_rep=np.ascontiguousarray(att_rep),
                  W_o=np.ascontiguousarray(Wo), iota=iota, ident_f=idf)
    in_maps = []
    for pl in plans:
        sl = np.zeros((M_PER_CORE, DIM), np.float32)
        nv = pl["n_valid"]
        sl[:nv] = ent[pl["node_lo"]:pl["node_lo"] + nv]
        m = dict(common)
        m["ent_slice"] = sl
        m["gidx"] = np.ascontiguousarray(np.stack(pl["gidx"], axis=1))
        m["tidx"] = pl["tidx"]
        m["shidx"] = pl["shidx"]
        m["offs"] = pl["offs"]
        in_maps.append(m)
    return in_maps


_CACHE = {}


def _get_nc(meta_key, meta, debug):
    key = (meta_key, debug)
    if key not in _CACHE:
        _CACHE[key] = _build_nc(meta, debug=debug)
    return _CACHE[key]


def run(inputs, debug=False, trace=False):
    plans, meta = _build_plan(np.asarray(inputs["edge_index"]),
                              np.asarray(inputs["edge_type"]))
    nc = _get_nc((meta["CPB"],), meta, debug)
    in_maps = _make_in_maps(inputs, plans)
    res = bass_utils.run_bass_kernel_spmd(nc, in_maps, core_ids=list(range(CORES)),
                                          trace=trace)
    out = np.zeros((N_ENT, DIM), np.float32)
    for c, pl in enumerate(plans):
        nv = pl["n_valid"]
        sl = np.asarray(res.results[c]["out_slice"])
        out[pl["node_lo"]:pl["node_lo"] + nv] = sl[:nv]
    return out, res, plans, meta


def kernel(**inputs) -> np.ndarray:
    out, _, _, _ = run(inputs)
    return out.astype(np.asarray(inputs["entity_embed"]).dtype)



# revision 11
# speedup vs baseline: 1.1300x; 1.1300x over previous
"""Trainium2 Bass kernel for nn_DAGNLinkPrediction (GNN message passing).

Self-contained: host-side integer preprocessing (sharding/permutation) + bass/tile
kernel + SPMD launch across 8 NeuronCores via run_bass_kernel_spmd.

Sharding: edges partitioned by src range (6272 nodes/core). Per core, edges are
sorted by (dst>=SPLIT, src-block, src) and padded into 128-edge chunks confined
to 128-node blocks. Per power iteration each core gathers Z[dst] rows (256B bf16)
with gpsimd.dma_gather, computes messages, segment-sums by src via PE matmuls
with 0/1 one-hot matrices, and AllGathers the updated bf16 node table.
"""
import numpy as np
import ml_dtypes

from concourse import bass, bacc, tile, bass_utils, mybir

BF = ml_dtypes.bfloat16
F32 = mybir.dt.float32
BF16 = mybir.dt.bfloat16
I16 = mybir.dt.int16

CORES = 8
N_ENT = 50000
N_REL = 500
HEADS = 2
DIM = 64
HD = HEADS * DIM                 # 128
M_PER_CORE = 6272                # 49*128
NBLK = M_PER_CORE // 128         # 49
TAB_ROWS = CORES * M_PER_CORE    # 50176
SPLIT = TAB_ROWS // 2            # 25088
NPIECE = 7                       # pieces per pass; NBLK = 7*7
BPP = NBLK // NPIECE             # blocks per piece = 7
ALPHA = 0.15
LN_EPS = 1e-5
L_LAYERS = 2
POW_ITER = 3
LRELU = 0.01

AF = mybir.ActivationFunctionType
OP = mybir.AluOpType


# ----------------------------------------------------------------------------
# host-side preprocessing (integer/layout only)
# ----------------------------------------------------------------------------

def _wrap_idxs(idx):
    n = idx.shape[0]
    base = idx.reshape(n // 16, 16).T.astype(np.int16)
    return np.ascontiguousarray(np.tile(base, (8, 1)))


def _build_plan(edge_index, edge_type):
    src = edge_index[0].astype(np.int64)
    dst = edge_index[1].astype(np.int64)
    et = edge_type.astype(np.int64)

    core_of = src // M_PER_CORE
    hi = (dst >= SPLIT).astype(np.int64)
    blk = (src % M_PER_CORE) // 128
    key = (core_of * 2 + hi) * NBLK + blk
    order = np.argsort(key, kind="stable")
    key_sorted = key[order]
    bounds = np.searchsorted(key_sorted, np.arange(CORES * 2 * NBLK + 1))

    counts = (bounds[1:] - bounds[:-1]).reshape(CORES, 2, NBLK)
    CPB = max(1, int(np.ceil(counts.max() / 128)))
    C_PASS = NBLK * CPB
    NIDX = C_PASS * 128

    plans = []
    for c in range(CORES):
        pl = {}
        pl["node_lo"] = c * M_PER_CORE
        pl["n_valid"] = max(0, min(N_ENT - c * M_PER_CORE, M_PER_CORE))
        gidx, offs, typs = [], [], []
        for p in range(2):
            eids = np.full(NIDX, -1, np.int64)
            for b in range(NBLK):
                k = (c * 2 + p) * NBLK + b
                lst = order[bounds[k]:bounds[k + 1]]
                eids[b * CPB * 128: b * CPB * 128 + len(lst)] = lst
            pad = eids < 0
            e_safe = np.where(pad, 0, eids)
            didx = dst[e_safe] - p * SPLIT
            didx[pad] = 0
            off = (src[e_safe] % 128).astype(np.int64)
            off[pad] = -1
            t = et[e_safe].copy()
            t[pad] = 0
            gidx.append(_wrap_idxs(didx.astype(np.int16)))
            offs.append(off.reshape(C_PASS, 128).T.astype(np.float32))  # [128, C_PASS]
            typs.append(t)
        pl["gidx"] = gidx
        pl["tidx"] = _wrap_idxs(np.concatenate(typs).astype(np.int16))
        shl = []
        for p in range(2):
            off_flat = offs[p].T.reshape(-1)  # [NIDX] slot-order src offsets (-1 pads)
            # rebuild local src ids from block index + offset
            blk_of_slot = np.repeat(np.arange(NBLK), CPB * 128)
            sl = blk_of_slot * 128 + np.maximum(off_flat, 0).astype(np.int64)
            sl[off_flat < 0] = 0
            shl.append(sl)
        pl["shidx"] = _wrap_idxs(np.concatenate(shl).astype(np.int16))
        pl["offs"] = np.concatenate(offs, axis=1).astype(BF)  # [128, 2*C_PASS]
        plans.append(pl)

    meta = dict(CPB=CPB, C_PASS=C_PASS, NIDX=NIDX,
                PIECE_CHUNKS=BPP * CPB, NIDX_PIECE=BPP * CPB * 128)
    return plans, meta


# ----------------------------------------------------------------------------
# device kernel
# ----------------------------------------------------------------------------

def _build_nc(meta, debug=False, n_layers=L_LAYERS, n_iters=POW_ITER, do_ag=True, do_node_ag=True):
    CPB = meta["CPB"]
    C_PASS = meta["C_PASS"]
    PC = meta["PIECE_CHUNKS"]          # chunks per piece
    NP_IDX = meta["NIDX_PIECE"]        # idxs per piece
    IW = meta["NIDX"] // 16            # idx cols per pass

    nc = bacc.Bacc("TRN2", target_bir_lowering=False, debug=False,
                   num_devices=CORES)

    # ---- external inputs ----
    ent_in = nc.dram_tensor("ent_slice", [M_PER_CORE, DIM], F32, kind="ExternalInput")
    rel_in = nc.dram_tensor("rel_emb", [512, DIM], F32, kind="ExternalInput")
    lng_in = nc.dram_tensor("ln_g", [128, L_LAYERS, DIM], F32, kind="ExternalInput")
    lnb_in = nc.dram_tensor("ln_b", [128, L_LAYERS, DIM], F32, kind="ExternalInput")
    W_in = nc.dram_tensor("W_htr", [3, L_LAYERS, DIM, HD], F32, kind="ExternalInput")
    att_in = nc.dram_tensor("att_rep", [3, L_LAYERS, 128, HD], F32, kind="ExternalInput")
    Wo_in = nc.dram_tensor("W_o", [L_LAYERS, HD, DIM], F32, kind="ExternalInput")
    gidx_in = nc.dram_tensor("gidx", [128, 2, IW], I16, kind="ExternalInput")
    tidx_in = nc.dram_tensor("tidx", [128, 2 * IW], I16, kind="ExternalInput")
    shidx_in = nc.dram_tensor("shidx", [128, 2 * IW], I16, kind="ExternalInput")
    offs_in = nc.dram_tensor("offs", [128, 2 * C_PASS], BF16, kind="ExternalInput")
    iota_in = nc.dram_tensor("iota", [128, 256], BF16, kind="ExternalInput")
    idf_in = nc.dram_tensor("ident_f", [128, 128], F32, kind="ExternalInput")

    out_ext = nc.dram_tensor("out_slice", [M_PER_CORE, DIM], F32, kind="ExternalOutput")
    if debug:
        dbg_at = nc.dram_tensor("dbg_at", [128, 2 * C_PASS, 2], BF16, kind="ExternalOutput")
        dbg_z0 = nc.dram_tensor("dbg_z0", [M_PER_CORE, HD], F32, kind="ExternalOutput")
        dbg_h = nc.dram_tensor("dbg_h", [M_PER_CORE, DIM], F32, kind="ExternalOutput")

    with tile.TileContext(nc) as tc:
        with tc.tile_pool(name="dram", bufs=1, space="DRAM") as dram, \
             tc.tile_pool(name="persist", bufs=1) as pp:
            table = dram.tile([TAB_ROWS, 128], BF16, tag="table")
            tab_in = dram.tile([M_PER_CORE, 128], BF16, tag="tab_in")
            srtab = dram.tile([512, 128], BF16, tag="srtab")
            shtab = dram.tile([M_PER_CORE, 128], BF16, tag="shtab")

            ent = pp.tile([128, NBLK, DIM], F32, tag="ent")
            h_t = pp.tile([128, NBLK, DIM], F32, tag="h")
            recip = pp.tile([128, NBLK, 2], F32, tag="recip")
            zt = pp.tile([128, NBLK, 2], F32, tag="zt")
            At = pp.tile([128, 2 * C_PASS, 2], BF16, tag="At")
            SRx = pp.tile([128, 2 * C_PASS, 8], BF16, tag="SRx")
            Z = pp.tile([128, NBLK, HD], F32, tag="Z")
            gidx = pp.tile([128, 2, IW], I16, tag="gidx")
            tidx = pp.tile([128, 2 * IW], I16, tag="tidx")
            shidx = pp.tile([128, 2 * IW], I16, tag="shidx")
            offs = pp.tile([128, 2 * C_PASS], BF16, tag="offs")
            iota = pp.tile([128, 256], BF16, tag="iota")
            idf = pp.tile([128, 128], F32, tag="idf")
            lng = pp.tile([128, L_LAYERS, DIM], F32, tag="lng")
            lnb = pp.tile([128, L_LAYERS, DIM], F32, tag="lnb")
            Wht = pp.tile([64, 3 * L_LAYERS, HD], F32, tag="Wht")
            attr = pp.tile([128, 3 * L_LAYERS, HD], F32, tag="attr")
            Wo = pp.tile([HD, L_LAYERS, DIM], F32, tag="Wo")

            # ---- load inputs ----
            nc.sync.dma_start(ent[:, :, :], ent_in.ap().rearrange("(b p) f -> p b f", p=128))
            nc.sync.dma_start(gidx[:, :, :], gidx_in.ap())
            nc.sync.dma_start(tidx[:, :], tidx_in.ap())
            nc.sync.dma_start(shidx[:, :], shidx_in.ap())
            nc.sync.dma_start(offs[:, :], offs_in.ap())
            nc.sync.dma_start(iota[:, :], iota_in.ap())
            nc.sync.dma_start(idf[:, :], idf_in.ap())
            nc.sync.dma_start(lng[:, :, :], lng_in.ap())
            nc.sync.dma_start(lnb[:, :, :], lnb_in.ap())
            nc.sync.dma_start(Wht[:, :, :], W_in.ap().rearrange("r l k m -> k (r l) m"))
            nc.sync.dma_start(attr[:, :, :], att_in.ap().rearrange("r l p m -> p (r l) m"))
            nc.sync.dma_start(Wo[:, :, :], Wo_in.ap().rearrange("l k m -> k l m"))

            def node_scores_block(pool, psum, lhsT, Wslice, att_ap, out_ap):
                """tanh(x@W) . att summed over d -> out_ap [128,2] (f32)."""
                pt = psum.tile([128, HD], F32, tag="ns_ps")
                nc.tensor.matmul(pt[:, :], lhsT, Wslice, start=True, stop=True)
                tt = pool.tile([128, HD], F32, tag="ns_tt")
                nc.scalar.activation(tt[:, :], pt[:, :], AF.Tanh)
                tm = pool.tile([128, HD], F32, tag="ns_tm")
                nc.vector.tensor_tensor(tm[:, :], tt[:, :], att_ap, OP.mult)
                nc.vector.tensor_reduce(out_ap, tm.rearrange("p (h d) -> p h d", h=2),
                                        mybir.AxisListType.X, OP.add)

            for l in range(n_layers):
                # ================= node phase =================
                with tc.tile_pool(name="nodep", bufs=2) as np_pool, \
                     tc.tile_pool(name="nodebig", bufs=1) as np_big, \
                     tc.tile_pool(name="nodeps", bufs=2, space="PSUM") as np_psum:
                    # layernorm
                    x = ent
                    mu = np_pool.tile([128, NBLK], F32, tag="mu")
                    nc.vector.tensor_reduce(mu[:, :], x[:, :, :], mybir.AxisListType.X, OP.add)
                    nc.vector.tensor_scalar(mu[:, :], mu[:, :], 1.0 / DIM, None, OP.mult)
                    xc = np_big.tile([128, NBLK, DIM], F32, tag="xc")
                    nc.vector.tensor_tensor(
                        xc[:, :, :], x[:, :, :],
                        mu.unsqueeze(2).broadcast_to([128, NBLK, DIM]), OP.subtract)
                    sq = np_big.tile([128, NBLK, DIM], F32, tag="sq")
                    nc.vector.tensor_tensor(sq[:, :, :], xc[:, :, :], xc[:, :, :], OP.mult)
                    var = np_pool.tile([128, NBLK], F32, tag="var")
                    nc.vector.tensor_reduce(var[:, :], sq[:, :, :], mybir.AxisListType.X, OP.add)
                    nc.vector.tensor_scalar(var[:, :], var[:, :], 1.0 / DIM, LN_EPS, OP.mult, OP.add)
                    std = np_pool.tile([128, NBLK], F32, tag="std")
                    nc.scalar.activation(std[:, :], var[:, :], AF.Sqrt)
                    rstd = np_pool.tile([128, NBLK], F32, tag="rstd")
                    nc.vector.reciprocal(rstd[:, :], std[:, :])
                    nc.vector.tensor_tensor(
                        h_t[:, :, :], xc[:, :, :],
                        rstd.unsqueeze(2).broadcast_to([128, NBLK, DIM]), OP.mult)
                    nc.vector.tensor_tensor(
                        h_t[:, :, :], h_t[:, :, :],
                        lng[:, l, :].unsqueeze(1).broadcast_to([128, NBLK, DIM]),
                        OP.mult)
                    nc.vector.tensor_tensor(
                        h_t[:, :, :], h_t[:, :, :],
                        lnb[:, l, :].unsqueeze(1).broadcast_to([128, NBLK, DIM]),
                        OP.add)
                    if debug and l == 0:
                        nc.sync.dma_start(dbg_h.ap().rearrange("(b p) f -> p b f", p=128),
                                          h_t[:, :, :])

                    # transpose h -> ht [64, b, 128]
                    ht = np_big.tile([64, NBLK, 128], F32, tag="ht")
                    for b in range(NBLK):
                        ps = np_psum.tile([64, 128], F32, tag="trh")
                        nc.tensor.transpose(ps[:, :], h_t[:, b, :], idf[:, :])
                        nc.scalar.activation(ht[:, b, :], ps[:, :], AF.Copy)

                    # s_h, s_t  [128, NBLK, 2] f32
                    s_f0 = np_pool.tile([128, NBLK, 2], F32, tag="s_f0")
                    s_f1 = np_pool.tile([128, NBLK, 2], F32, tag="s_f1")
                    s_f = [s_f0, s_f1]
                    for r in range(2):
                        for b in range(NBLK):
                            node_scores_block(np_pool, np_psum, ht[:, b, :],
                                              Wht[:, r * L_LAYERS + l, :],
                                              attr[:, r * L_LAYERS + l, :],
                                              s_f[r][:, b, :])
                    # sh table rows: [bf16(s_h)(2), bf16(residual)(2), 0...]
                    shsb = np_big.tile([128, NBLK, 128], BF16, tag="shsb")
                    shm_f = np_pool.tile([128, NBLK, 2], F32, tag="shm_f")
                    nc.vector.memset(shsb[:, :, 4:128], 0.0)
                    nc.vector.tensor_copy(shsb[:, :, 0:2], s_f[0][:, :, :])
                    nc.vector.tensor_copy(shm_f[:, :, :], shsb[:, :, 0:2])
                    nc.vector.tensor_tensor(shsb[:, :, 2:4], s_f[0][:, :, :], shm_f[:, :, :],
                                            OP.subtract)
                    nc.sync.dma_start(shtab.rearrange("(b p) c -> p b c", p=128),
                                      shsb[:, :, :])

                    # gather table slice: [h | s_t | s_t_res | 1 | 0...]
                    tabsb = np_big.tile([128, NBLK, 128], BF16, tag="tabsb")
                    nc.vector.tensor_copy(tabsb[:, :, 0:64], h_t[:, :, :])
                    nc.vector.tensor_copy(tabsb[:, :, 64:66], s_f[1][:, :, :])
                    stm_f = np_pool.tile([128, NBLK, 2], F32, tag="stm_f")
                    nc.vector.tensor_copy(stm_f[:, :, :], tabsb[:, :, 64:66])
                    nc.vector.tensor_tensor(tabsb[:, :, 66:68], s_f[1][:, :, :], stm_f[:, :, :],
                                            OP.subtract)
                    nc.vector.memset(tabsb[:, :, 68:69], 1.0)
                    nc.vector.memset(tabsb[:, :, 69:128], 0.0)
                    nc.sync.dma_start(tab_in.rearrange("(b p) c -> p b c", p=128),
                                      tabsb[:, :, :])
                    if do_node_ag:
                        nc.gpsimd.collective_compute(
                            "AllGather", OP.bypass,
                            replica_groups=[list(range(CORES))],
                            ins=[tab_in.opt()], outs=[table.opt()])
                    else:
                        nc.sync.dma_start(table[0:M_PER_CORE, :],
                                          tab_in[:, :])

                    # s_r table (once, both layers)
                    if l == 0:
                        relsb = np_pool.tile([128, 4, DIM], F32, tag="relsb")
                        nc.sync.dma_start(relsb[:, :, :],
                                          rel_in.ap().rearrange("(b p) f -> p b f", p=128))
                        relt = np_pool.tile([64, 4, 128], F32, tag="relt")
                        for b in range(4):
                            ps = np_psum.tile([64, 128], F32, tag="trh")
                            nc.tensor.transpose(ps[:, :], relsb[:, b, :], idf[:, :])
                            nc.scalar.activation(relt[:, b, :], ps[:, :], AF.Copy)
                        srsb = np_pool.tile([128, 4, 128], BF16, tag="srsb")
                        nc.vector.memset(srsb[:, :, :], 0.0)
                        sr_f = np_pool.tile([128, 4, 2 * L_LAYERS], F32, tag="sr_f")
                        for ll in range(L_LAYERS):
                            for b in range(4):
                                node_scores_block(np_pool, np_psum, relt[:, b, :],
                                                  Wht[:, 2 * L_LAYERS + ll, :],
                                                  attr[:, 2 * L_LAYERS + ll, :],
                                                  sr_f[:, b, 2 * ll:2 * ll + 2])
                        nc.vector.tensor_copy(srsb[:, :, 0:4], sr_f[:, :, :])
                        srm_f = np_pool.tile([128, 4, 4], F32, tag="srm_f")
                        nc.vector.tensor_copy(srm_f[:, :, :], srsb[:, :, 0:4])
                        nc.vector.tensor_tensor(srsb[:, :, 4:8], sr_f[:, :, :], srm_f[:, :, :],
                                                OP.subtract)
                        nc.sync.dma_start(srtab.rearrange("(b p) c -> p b c", p=128),
                                          srsb[:, :, :])

                # ================= edge phase =================
                for it in range(n_iters):
                    with tc.tile_pool(name="edgep", bufs=3) as ep, \
                         tc.tile_pool(name="edgeps", bufs=2) as eps, \
                         tc.tile_pool(name="edgep2", bufs=2) as ep2, \
                         tc.tile_pool(name="spmm_ps", bufs=2, space="PSUM") as spmm_ps, \
                         tc.tile_pool(name="zps_pool", bufs=2, space="PSUM") as zps_pool:
                        for k in range(NPIECE):
                            psb = spmm_ps.tile([128, BPP, 128], F32, tag="blkps")
                            psz = zps_pool.tile([128, BPP, 2], F32, tag="zps")
                            for p in range(2):
                                slot0 = p * C_PASS + k * PC
                                Gt = ep.tile([128, PC, 128], BF16, tag="Gt")
                                nc.gpsimd.dma_gather(
                                    out_ap=Gt[:, :, :],
                                    in_ap=table[p * SPLIT:, :],
                                    idxs_ap=gidx[:, p, k * (NP_IDX // 16):(k + 1) * (NP_IDX // 16)],
                                    num_idxs=NP_IDX, num_idxs_reg=NP_IDX, elem_size=128, single_packet=False)
                                # transposed one-hot [128e, 128n, c]; rebuilt on
                                # DVE every iteration (2x 16-bit mode via (j2,k)
                                # pair views) — cheaper than DRAM caching.
                                oh_t = ep2.tile([128, 128, PC], BF16, tag="oh")
                                nc.vector.tensor_tensor(
                                    oh_t.rearrange("p n (j2 e) -> p n j2 e", e=2),
                                    offs[:, slot0:slot0 + PC]
                                        .rearrange("p (j2 e) -> p j2 e", e=2)
                                        .unsqueeze(1)
                                        .broadcast_to([128, 128, PC // 2, 2]),
                                    iota.rearrange("p (n e) -> p n e", e=2)
                                        .unsqueeze(2)
                                        .broadcast_to([128, 128, PC // 2, 2]),
                                    OP.is_equal)

                                if it == 0:
                                    idx0 = (p * NPIECE + k) * (NP_IDX // 16)
                                    idx1 = (p * NPIECE + k + 1) * (NP_IDX // 16)
                                    if l == 0:
                                        SRt = eps.tile([128, PC, 128], BF16, tag="SRt")
                                        nc.gpsimd.dma_gather(
                                            out_ap=SRt[:, :, :],
                                            in_ap=srtab[:, :],
                                            idxs_ap=tidx[:, idx0:idx1],
                                            num_idxs=NP_IDX, num_idxs_reg=NP_IDX, elem_size=128, single_packet=False)
                                        nc.vector.tensor_copy(
                                            SRx[:, slot0:slot0 + PC, :], SRt[:, :, 0:8])
                                    SHt = eps.tile([128, PC, 128], BF16, tag="SHt")
                                    nc.gpsimd.dma_gather(
                                        out_ap=SHt[:, :, :],
                                        in_ap=shtab[:, :],
                                        idxs_ap=shidx[:, idx0:idx1],
                                        num_idxs=NP_IDX, num_idxs_reg=NP_IDX, elem_size=128, single_packet=False)
                                    # scores
                                    sc = ep2.tile([128, PC, 2], F32, tag="sc")
                                    nc.vector.tensor_tensor(sc[:, :, :], Gt[:, :, 64:66],
                                                            Gt[:, :, 66:68], OP.add)
                                    t2 = ep2.tile([128, PC, 2], F32, tag="t2")
                                    nc.vector.tensor_tensor(
                                        t2[:, :, :],
                                        SRx[:, slot0:slot0 + PC, 2 * l:2 * l + 2],
                                        SRx[:, slot0:slot0 + PC, 4 + 2 * l:6 + 2 * l], OP.add)
                                    nc.vector.tensor_tensor(sc[:, :, :], sc[:, :, :], t2[:, :, :],
                                                            OP.add)
                                    nc.vector.tensor_tensor(t2[:, :, :], SHt[:, :, 0:2],
                                                            SHt[:, :, 2:4], OP.add)
                                    nc.vector.tensor_tensor(sc[:, :, :], sc[:, :, :], t2[:, :, :],
                                                            OP.add)
                                    nc.vector.scalar_tensor_tensor(
                                        sc[:, :, :], sc[:, :, :], LRELU, sc[:, :, :],
                                        OP.mult, OP.max)
                                    nc.scalar.activation(At[:, slot0:slot0 + PC, :],
                                                         sc[:, :, :], AF.Exp)

                                # messages, interleaved cols (c = 2d+h).
                                A4 = At[:, slot0:slot0 + PC, :].unsqueeze(2) \
                                    .broadcast_to([128, PC, 64, 2])
                                if it == 0:
                                    # Z0 = h (cols 0:64) for both heads
                                    msg = ep.tile([128, PC, 128], BF16, tag="Gt")
                                    nc.vector.tensor_tensor(
                                        msg.rearrange("p c (d h) -> p c d h", h=2),
                                        Gt[:, :, 0:64].unsqueeze(3)
                                            .broadcast_to([128, PC, 64, 2]),
                                        A4, OP.mult)
                                else:
                                    # in place: same column elementwise (2x DVE)
                                    msg = Gt
                                    G4 = Gt.rearrange("p c (d h) -> p c d h", h=2)
                                    nc.vector.tensor_tensor(G4, G4, A4, OP.mult)
                                # spmm (+ z columns via separate 2-col matmul on iter 0)
                                # PSUM zero-regions are whole banks: start/stop once per bank.
                                bank1_j0 = 4 * CPB
                                for j in range(PC):
                                    bl = j // CPB
                                    st = (p == 0) and (j == 0 or j == bank1_j0)
                                    sp = (p == 1) and (j == bank1_j0 - 1 or j == PC - 1)
                                    nc.tensor.matmul(
                                        psb[:, bl, :], oh_t[:, :, j], msg[:, j, :],
                                        start=st, stop=sp)
                                    if it == 0:
                                        nc.tensor.matmul(
                                            psz[:, bl, :], oh_t[:, :, j],
                                            At[:, slot0 + j, :],
                                            start=(p == 0 and j == 0),
                                            stop=(p == 1 and j == PC - 1))

                            # piece epilogue: z, recip, Z assembly
                            if it == 0:
                                b0 = k * BPP
                                nc.vector.tensor_scalar(zt[:, b0:b0 + BPP, :], psz[:, :, :],
                                                        1e-30, None, OP.max)
                                nc.vector.reciprocal(recip[:, b0:b0 + BPP, :],
                                                     zt[:, b0:b0 + BPP, :])
                                nc.vector.tensor_scalar(recip[:, b0:b0 + BPP, :],
                                                        recip[:, b0:b0 + BPP, :],
                                                        1.0 - ALPHA, None, OP.mult)
                            for bl in range(BPP):
                                b = k * BPP + bl
                                Z4b = Z[:, b, :].rearrange("p (d h) -> p d h", h=2)
                                psb4 = psb[:, bl, :].rearrange("p (d h) -> p d h", h=2)
                                nc.scalar.activation(Z4b[:, :, 0], psb4[:, :, 0], AF.Copy,
                                                     scale=recip[:, b, 0:1])
                                nc.scalar.activation(Z4b[:, :, 1], psb4[:, :, 1],
                                                     AF.Copy, scale=recip[:, b, 1:2])
                                nc.vector.scalar_tensor_tensor(
                                    Z4b,
                                    h_t[:, b, :].unsqueeze(2)
                                        .broadcast_to([128, 64, 2]),
                                    ALPHA, Z4b,
                                    OP.mult, OP.add)

                        if debug and l == 0 and it == 0:
                            nc.sync.dma_start(dbg_z0.ap().rearrange("(b p) c -> p b c", p=128),
                                              Z[:, :, :])
                            nc.sync.dma_start(
                                dbg_at.ap(),
                                At[:, :, :])

                        if it < n_iters - 1:
                            tabz = ep.tile([128, NBLK, 128], BF16, tag="Gt")
                            nc.vector.tensor_copy(tabz[:, :, :], Z[:, :, :])
                            nc.sync.dma_start(tab_in.rearrange("(b p) c -> p b c", p=128),
                                              tabz[:, :, :])
                            if do_ag:
                                nc.gpsimd.collective_compute(
                                    "AllGather", OP.bypass,
                                    replica_groups=[list(range(CORES))],
                                    ins=[tab_in.opt()], outs=[table.opt()])

                # ================= conv + residual =================
                with tc.tile_pool(name="convp", bufs=2) as cp, \
                     tc.tile_pool(name="convps", bufs=4, space="PSUM") as cps:
                    for b in range(NBLK):
                        pzt = cps.tile([128, 128], F32, tag="pzt")
                        nc.tensor.transpose(pzt[:, :], Z[:, b, :], idf[:, :])
                        Zt = cp.tile([128, 128], F32, tag="Zt")
                        nc.scalar.activation(Zt[:, :], pzt[:, :], AF.Copy)
                        pc_ = cps.tile([128, 64], F32, tag="pc")
                        nc.tensor.matmul(pc_[:, :], Zt[:, :], Wo[:, l, :],
                                         start=True, stop=True)
                        nc.vector.tensor_tensor(ent[:, b, :], pc_[:, :], ent[:, b, :], OP.add)

            nc.sync.dma_start(out_ext.ap().rearrange("(b p) f -> p b f", p=128),
                              ent[:, :, :])

    nc.compile()
    return nc


# ----------------------------------------------------------------------------
# host wrapper
# ----------------------------------------------------------------------------

def _make_in_maps(inputs, plans):
    ent = np.asarray(inputs["entity_embed"], np.float32)
    rel = np.zeros((512, DIM), np.float32)
    rel[:N_REL] = np.asarray(inputs["relation_embed"], np.float32)
    lng = np.tile(np.asarray(inputs["ln_gamma"], np.float32)[None], (128, 1, 1))
    lnb = np.tile(np.asarray(inputs["ln_beta"], np.float32)[None], (128, 1, 1))
    W = np.stack([np.asarray(inputs["W_h"], np.float32),
                  np.asarray(inputs["W_t"], np.float32),
                  np.asarray(inputs["W_r"], np.float32)], axis=0)
    att = np.stack([np.asarray(inputs["att_h"], np.float32),
                    np.asarray(inputs["att_t"], np.float32),
                    np.asarray(inputs["att_r"], np.float32)], axis=0)
    att_rep = np.tile(att.reshape(3, L_LAYERS, 1, HD), (1, 1, 128, 1)).astype(np.float32)
    # Z columns are interleaved as c = 2d+h on device; permute W_o rows to match
    cperm = (np.arange(128) % 2) * 64 + np.arange(128) // 2
    Wo = np.ascontiguousarray(np.asarray(inputs["W_o"], np.float32)[:, cperm, :])
    iota = np.tile(np.repeat(np.arange(128, dtype=np.float32), 2)[None],
                   (128, 1)).astype(BF)
    idf = np.eye(128, dtype=np.float32)

    common = dict(rel_emb=rel, ln_g=np.ascontiguousarray(lng), ln_b=np.ascontiguousarray(lnb),
                  W_htr=np.ascontiguousarray(W), att_rep=np.ascontiguousarray(att_rep),
                  W_o=np.ascontiguousarray(Wo), iota=iota, ident_f=idf)
    in_maps = []
    for pl in plans:
        sl = np.zeros((M_PER_CORE, DIM), np.float32)
        nv = pl["n_valid"]
        sl[:nv] = ent[pl["node_lo"]:pl["node_lo"] + nv]
        m = dict(common)
        m["ent_slice"] = sl
        m["gidx"] = np.ascontiguousarray(np.stack(pl["gidx"], axis=1))
        m["tidx"] = pl["tidx"]
        m["shidx"] = pl["shidx"]
        m["offs"] = pl["offs"]
        in_maps.append(m)
    return in_maps


_CACHE = {}


def _get_nc(meta_key, meta, debug):
    key = (meta_key, debug)
    if key not in _CACHE:
        _CACHE[key] = _build_nc(meta, debug=debug)
    return _CACHE[key]


def run(inputs, debug=False, trace=False):
    plans, meta = _build_plan(np.asarray(inputs["edge_index"]),
                              np.asarray(inputs["edge_type"]))
    nc = _get_nc((meta["CPB"],), meta, debug)
    in_maps = _make_in_maps(inputs, plans)
    res = bass_utils.run_bass_kernel_spmd(nc, in_maps, core_ids=list(range(CORES)),
                                          trace=trace)
    out = np.zeros((N_ENT, DIM), np.float32)
    for c, pl in enumerate(plans):
        nv = pl["n_valid"]
        sl = np.asarray(res.results[c]["out_slice"])
        out[pl["node_lo"]:pl["node_lo"] + nv] = sl[:nv]
    return out, res, plans, meta


def kernel(**inputs) -> np.ndarray:
    out, _, _, _ = run(inputs)
    return out.astype(np.asarray(inputs["entity_embed"]).dtype)



# revision 16
# speedup vs baseline: 1.2510x; 1.1071x over previous
"""Trainium2 Bass kernel for nn_DAGNLinkPrediction (GNN message passing).

Self-contained: host-side integer preprocessing (sharding/permutation) + bass/tile
kernel + SPMD launch across 8 NeuronCores via run_bass_kernel_spmd.

Sharding: edges partitioned by src range (6272 nodes/core). Per core, edges are
sorted by (dst>=SPLIT, src-block, src) and padded into 128-edge chunks confined
to 128-node blocks. Per power iteration each core gathers Z[dst] rows (256B bf16)
with gpsimd.dma_gather, computes messages, segment-sums by src via PE matmuls
with 0/1 one-hot matrices, and AllGathers the updated bf16 node table.
"""
import numpy as np
import ml_dtypes

from concourse import bass, bacc, tile, bass_utils, mybir

BF = ml_dtypes.bfloat16
F32 = mybir.dt.float32
BF16 = mybir.dt.bfloat16
I16 = mybir.dt.int16

CORES = 8
N_ENT = 50000
N_REL = 500
HEADS = 2
DIM = 64
HD = HEADS * DIM                 # 128
M_PER_CORE = 6272                # 49*128
NBLK = M_PER_CORE // 128         # 49
TAB_ROWS = CORES * M_PER_CORE    # 50176
SPLIT = TAB_ROWS // 2            # 25088
NPIECE = 7                       # pieces per pass; NBLK = 7*7
BPP = NBLK // NPIECE             # blocks per piece = 7
ALPHA = 0.15
LN_EPS = 1e-5
L_LAYERS = 2
POW_ITER = 3
LRELU = 0.01
N_EDGE_KEY = 8192                # > M_PER_CORE; packs (key, src pos) for sort

AF = mybir.ActivationFunctionType
OP = mybir.AluOpType


# ----------------------------------------------------------------------------
# host-side preprocessing (integer/layout only)
# ----------------------------------------------------------------------------

def _wrap_idxs(idx):
    n = idx.shape[0]
    base = idx.reshape(n // 16, 16).T.astype(np.int16)
    return np.ascontiguousarray(np.tile(base, (8, 1)))


def _pack_core(deg, caps):
    """Assign 6272 nodes (deg: [6272, 2] edge counts per dst-half) to 49
    blocks of exactly 128 nodes with per-block per-half edge capacity caps[b].
    Returns blk_of [6272] or None if infeasible."""
    n_nodes = deg.shape[0]
    order = np.argsort(-(deg.sum(1) * 64 + deg.max(1)), kind="stable")
    slots = np.full(NBLK, 128, np.int64)
    load = np.zeros((NBLK, 2), np.int64)
    blk_of = np.empty(n_nodes, np.int64)
    capsf = caps.astype(np.float64)
    for n in order:
        d0, d1 = deg[n]
        sc = np.maximum(load[:, 0] + d0, load[:, 1] + d1) / capsf
        sc[slots <= 0] = np.inf
        b = int(np.argmin(sc))
        if sc[b] > 1.0:
            return None
        blk_of[n] = b
        slots[b] -= 1
        load[b] += deg[n]
    return blk_of


def _build_plan(edge_index, edge_type):
    src = edge_index[0].astype(np.int64)
    dst = edge_index[1].astype(np.int64)
    et = edge_type.astype(np.int64)

    src_core = src // M_PER_CORE
    src_loc = src % M_PER_CORE
    hi = (dst >= SPLIT).astype(np.int64)

    # per-core, per-half node degrees
    deg = np.zeros((CORES, M_PER_CORE, 2), np.int64)
    np.add.at(deg, (src_core, src_loc, hi), 1)

    # find the smallest even per-piece chunk count with a feasible packing
    blk_of_all = None
    for PCP in range(36, 44, 2):
        prof_piece = [5] * NPIECE
        for i in range(PCP - 5 * NPIECE):
            prof_piece[i % NPIECE] += 1
        prof = np.array(prof_piece * NPIECE, np.int64)  # [NBLK]
        caps = prof * 128
        blk_of_all = []
        ok = True
        for c in range(CORES):
            blk_of = _pack_core(deg[c], caps)
            if blk_of is None:
                ok = False
                break
            blk_of_all.append(blk_of)
        if ok:
            break
    assert blk_of_all is not None and len(blk_of_all) == CORES, "packing failed"

    NCH = int(prof.sum())                     # chunks per pass
    NIDX = NCH * 128
    chunk0 = np.concatenate([[0], np.cumsum(prof)])  # chunk offset per block

    # new position of every node: newpos = blk*128 + slot (stable within block)
    newpos = np.empty((CORES, M_PER_CORE), np.int64)
    perms = []
    for c in range(CORES):
        order_nodes = np.argsort(blk_of_all[c] * M_PER_CORE
                                 + np.arange(M_PER_CORE), kind="stable")
        # order_nodes[i] = old local id placed at new position i
        perms.append(order_nodes)
        inv = np.empty(M_PER_CORE, np.int64)
        inv[order_nodes] = np.arange(M_PER_CORE)
        newpos[c] = inv
    dst_new = (dst // M_PER_CORE) * M_PER_CORE + newpos[dst // M_PER_CORE,
                                                        dst % M_PER_CORE]
    src_new = src_core * M_PER_CORE + newpos[src_core, src_loc]
    src_nloc = src_new % M_PER_CORE
    blk = src_nloc // 128

    key = (src_core * 2 + hi) * NBLK + blk
    sortkey = key * N_EDGE_KEY + src_nloc
    order = np.argsort(sortkey, kind="stable")
    bounds = np.searchsorted(sortkey[order],
                             np.arange(CORES * 2 * NBLK + 1) * N_EDGE_KEY,
                             side="left")

    plans = []
    for c in range(CORES):
        pl = {}
        pl["node_lo"] = c * M_PER_CORE
        pl["n_valid"] = max(0, min(N_ENT - c * M_PER_CORE, M_PER_CORE))
        pl["perm"] = perms[c]
        gidx, offs, typs, shl = [], [], [], []
        for p in range(2):
            eids = np.full(NIDX, -1, np.int64)
            for b in range(NBLK):
                k = (c * 2 + p) * NBLK + b
                lst = order[bounds[k]:bounds[k + 1]]
                s0 = chunk0[b] * 128
                assert len(lst) <= (chunk0[b + 1] - chunk0[b]) * 128
                eids[s0:s0 + len(lst)] = lst
            pad = eids < 0
            e_safe = np.where(pad, 0, eids)
            didx = dst_new[e_safe] - p * SPLIT
            didx[pad] = 0
            off = (src_nloc[e_safe] % 128)
            off[pad] = -1
            t = et[e_safe].copy()
            t[pad] = 0
            sl = src_nloc[e_safe].copy()
            sl[pad] = 0
            gidx.append(_wrap_idxs(didx.astype(np.int16)))
            offs.append(off.reshape(NCH, 128).T.astype(np.float32))  # [128, NCH]
            typs.append(t)
            shl.append(sl)
        pl["gidx"] = gidx
        pl["tidx"] = _wrap_idxs(np.concatenate(typs).astype(np.int16))
        pl["shidx"] = _wrap_idxs(np.concatenate(shl).astype(np.int16))
        pl["offs"] = np.concatenate(offs, axis=1).astype(BF)  # [128, 2*NCH]
        plans.append(pl)

    j2bl = []
    for b, cnt in enumerate(prof_piece):
        j2bl += [b] * cnt
    bank1_j0 = next(j for j, b in enumerate(j2bl) if b >= 4)
    meta = dict(C_PASS=NCH, NIDX=NIDX, PIECE_CHUNKS=PCP,
                NIDX_PIECE=PCP * 128, J2BL=tuple(j2bl), BANK1_J0=bank1_j0)
    return plans, meta


# ----------------------------------------------------------------------------
# device kernel
# ----------------------------------------------------------------------------

def _build_nc(meta, debug=False, n_layers=L_LAYERS, n_iters=POW_ITER, do_ag=True, do_node_ag=True):
    C_PASS = meta["C_PASS"]
    J2BL = meta["J2BL"]
    PC = meta["PIECE_CHUNKS"]          # chunks per piece
    NP_IDX = meta["NIDX_PIECE"]        # idxs per piece
    IW = meta["NIDX"] // 16            # idx cols per pass

    nc = bacc.Bacc("TRN2", target_bir_lowering=False, debug=False,
                   num_devices=CORES)

    # ---- external inputs ----
    ent_in = nc.dram_tensor("ent_slice", [M_PER_CORE, DIM], F32, kind="ExternalInput")
    rel_in = nc.dram_tensor("rel_emb", [512, DIM], F32, kind="ExternalInput")
    lng_in = nc.dram_tensor("ln_g", [128, L_LAYERS, DIM], F32, kind="ExternalInput")
    lnb_in = nc.dram_tensor("ln_b", [128, L_LAYERS, DIM], F32, kind="ExternalInput")
    W_in = nc.dram_tensor("W_htr", [3, L_LAYERS, DIM, HD], F32, kind="ExternalInput")
    att_in = nc.dram_tensor("att_rep", [3, L_LAYERS, 128, HD], F32, kind="ExternalInput")
    Wo_in = nc.dram_tensor("W_o", [L_LAYERS, HD, DIM], F32, kind="ExternalInput")
    gidx_in = nc.dram_tensor("gidx", [128, 2, IW], I16, kind="ExternalInput")
    tidx_in = nc.dram_tensor("tidx", [128, 2 * IW], I16, kind="ExternalInput")
    shidx_in = nc.dram_tensor("shidx", [128, 2 * IW], I16, kind="ExternalInput")
    offs_in = nc.dram_tensor("offs", [128, 2 * C_PASS], BF16, kind="ExternalInput")
    iota_in = nc.dram_tensor("iota", [128, 256], BF16, kind="ExternalInput")
    idf_in = nc.dram_tensor("ident_f", [128, 128], F32, kind="ExternalInput")

    out_ext = nc.dram_tensor("out_slice", [M_PER_CORE, DIM], F32, kind="ExternalOutput")
    if debug:
        dbg_at = nc.dram_tensor("dbg_at", [128, 2 * C_PASS, 2], BF16, kind="ExternalOutput")
        dbg_z0 = nc.dram_tensor("dbg_z0", [M_PER_CORE, HD], F32, kind="ExternalOutput")
        dbg_h = nc.dram_tensor("dbg_h", [M_PER_CORE, DIM], F32, kind="ExternalOutput")

    with tile.TileContext(nc) as tc:
        with tc.tile_pool(name="dram", bufs=1, space="DRAM") as dram, \
             tc.tile_pool(name="persist", bufs=1) as pp:
            table = dram.tile([TAB_ROWS, 128], BF16, tag="table")
            tab_in = dram.tile([M_PER_CORE, 128], BF16, tag="tab_in")
            srtab = dram.tile([512, 128], BF16, tag="srtab")
            shtab = dram.tile([M_PER_CORE, 128], BF16, tag="shtab")

            ent = pp.tile([128, NBLK, DIM], F32, tag="ent")
            h_t = pp.tile([128, NBLK, DIM], F32, tag="h")
            recip = pp.tile([128, NBLK, 2], F32, tag="recip")
            zt = pp.tile([128, NBLK, 2], F32, tag="zt")
            At = pp.tile([128, 2 * C_PASS, 2], BF16, tag="At")
            SRx = pp.tile([128, 2 * C_PASS, 8], BF16, tag="SRx")
            Z = pp.tile([128, NBLK, HD], F32, tag="Z")
            gidx = pp.tile([128, 2, IW], I16, tag="gidx")
            tidx = pp.tile([128, 2 * IW], I16, tag="tidx")
            shidx = pp.tile([128, 2 * IW], I16, tag="shidx")
            offs = pp.tile([128, 2 * C_PASS], BF16, tag="offs")
            iota = pp.tile([128, 256], BF16, tag="iota")
            idf = pp.tile([128, 128], F32, tag="idf")
            lng = pp.tile([128, L_LAYERS, DIM], F32, tag="lng")
            lnb = pp.tile([128, L_LAYERS, DIM], F32, tag="lnb")
            Wht = pp.tile([64, 3 * L_LAYERS, HD], F32, tag="Wht")
            attr = pp.tile([128, 3 * L_LAYERS, HD], F32, tag="attr")
            Wo = pp.tile([HD, L_LAYERS, DIM], F32, tag="Wo")

            # ---- load inputs ----
            nc.sync.dma_start(ent[:, :, :], ent_in.ap().rearrange("(b p) f -> p b f", p=128))
            nc.sync.dma_start(gidx[:, :, :], gidx_in.ap())
            nc.sync.dma_start(tidx[:, :], tidx_in.ap())
            nc.sync.dma_start(shidx[:, :], shidx_in.ap())
            nc.sync.dma_start(offs[:, :], offs_in.ap())
            nc.sync.dma_start(iota[:, :], iota_in.ap())
            nc.sync.dma_start(idf[:, :], idf_in.ap())
            nc.sync.dma_start(lng[:, :, :], lng_in.ap())
            nc.sync.dma_start(lnb[:, :, :], lnb_in.ap())
            nc.sync.dma_start(Wht[:, :, :], W_in.ap().rearrange("r l k m -> k (r l) m"))
            nc.sync.dma_start(attr[:, :, :], att_in.ap().rearrange("r l p m -> p (r l) m"))
            nc.sync.dma_start(Wo[:, :, :], Wo_in.ap().rearrange("l k m -> k l m"))

            def node_scores_block(pool, psum, lhsT, Wslice, att_ap, out_ap):
                """tanh(x@W) . att summed over d -> out_ap [128,2] (f32)."""
                pt = psum.tile([128, HD], F32, tag="ns_ps")
                nc.tensor.matmul(pt[:, :], lhsT, Wslice, start=True, stop=True)
                tt = pool.tile([128, HD], F32, tag="ns_tt")
                nc.scalar.activation(tt[:, :], pt[:, :], AF.Tanh)
                tm = pool.tile([128, HD], F32, tag="ns_tm")
                nc.vector.tensor_tensor(tm[:, :], tt[:, :], att_ap, OP.mult)
                nc.vector.tensor_reduce(out_ap, tm.rearrange("p (h d) -> p h d", h=2),
                                        mybir.AxisListType.X, OP.add)

            for l in range(n_layers):
                # ================= node phase =================
                with tc.tile_pool(name="nodep", bufs=2) as np_pool, \
                     tc.tile_pool(name="nodebig", bufs=1) as np_big, \
                     tc.tile_pool(name="nodeps", bufs=2, space="PSUM") as np_psum:
                    # layernorm
                    x = ent
                    mu = np_pool.tile([128, NBLK], F32, tag="mu")
                    nc.vector.tensor_reduce(mu[:, :], x[:, :, :], mybir.AxisListType.X, OP.add)
                    nc.vector.tensor_scalar(mu[:, :], mu[:, :], 1.0 / DIM, None, OP.mult)
                    xc = np_big.tile([128, NBLK, DIM], F32, tag="xc")
                    nc.vector.tensor_tensor(
                        xc[:, :, :], x[:, :, :],
                        mu.unsqueeze(2).broadcast_to([128, NBLK, DIM]), OP.subtract)
                    sq = np_big.tile([128, NBLK, DIM], F32, tag="sq")
                    nc.vector.tensor_tensor(sq[:, :, :], xc[:, :, :], xc[:, :, :], OP.mult)
                    var = np_pool.tile([128, NBLK], F32, tag="var")
                    nc.vector.tensor_reduce(var[:, :], sq[:, :, :], mybir.AxisListType.X, OP.add)
                    nc.vector.tensor_scalar(var[:, :], var[:, :], 1.0 / DIM, LN_EPS, OP.mult, OP.add)
                    std = np_pool.tile([128, NBLK], F32, tag="std")
                    nc.scalar.activation(std[:, :], var[:, :], AF.Sqrt)
                    rstd = np_pool.tile([128, NBLK], F32, tag="rstd")
                    nc.vector.reciprocal(rstd[:, :], std[:, :])
                    nc.vector.tensor_tensor(
                        h_t[:, :, :], xc[:, :, :],
                        rstd.unsqueeze(2).broadcast_to([128, NBLK, DIM]), OP.mult)
                    nc.vector.tensor_tensor(
                        h_t[:, :, :], h_t[:, :, :],
                        lng[:, l, :].unsqueeze(1).broadcast_to([128, NBLK, DIM]),
                        OP.mult)
                    nc.vector.tensor_tensor(
                        h_t[:, :, :], h_t[:, :, :],
                        lnb[:, l, :].unsqueeze(1).broadcast_to([128, NBLK, DIM]),
                        OP.add)
                    if debug and l == 0:
                        nc.sync.dma_start(dbg_h.ap().rearrange("(b p) f -> p b f", p=128),
                                          h_t[:, :, :])

                    # transpose h -> ht [64, b, 128]
                    ht = np_big.tile([64, NBLK, 128], F32, tag="ht")
                    for b in range(NBLK):
                        ps = np_psum.tile([64, 128], F32, tag="trh")
                        nc.tensor.transpose(ps[:, :], h_t[:, b, :], idf[:, :])
                        nc.scalar.activation(ht[:, b, :], ps[:, :], AF.Copy)

                    # s_h, s_t  [128, NBLK, 2] f32
                    s_f0 = np_pool.tile([128, NBLK, 2], F32, tag="s_f0")
                    s_f1 = np_pool.tile([128, NBLK, 2], F32, tag="s_f1")
                    s_f = [s_f0, s_f1]
                    for r in range(2):
                        for b in range(NBLK):
                            node_scores_block(np_pool, np_psum, ht[:, b, :],
                                              Wht[:, r * L_LAYERS + l, :],
                                              attr[:, r * L_LAYERS + l, :],
                                              s_f[r][:, b, :])
                    # sh table rows: [bf16(s_h)(2), bf16(residual)(2), 0...]
                    shsb = np_big.tile([128, NBLK, 128], BF16, tag="shsb")
                    shm_f = np_pool.tile([128, NBLK, 2], F32, tag="shm_f")
                    nc.vector.memset(shsb[:, :, 4:128], 0.0)
                    nc.vector.tensor_copy(shsb[:, :, 0:2], s_f[0][:, :, :])
                    nc.vector.tensor_copy(shm_f[:, :, :], shsb[:, :, 0:2])
                    nc.vector.tensor_tensor(shsb[:, :, 2:4], s_f[0][:, :, :], shm_f[:, :, :],
                                            OP.subtract)
                    nc.sync.dma_start(shtab.rearrange("(b p) c -> p b c", p=128),
                                      shsb[:, :, :])

                    # gather table slice: [h | s_t | s_t_res | 1 | 0...]
                    tabsb = np_big.tile([128, NBLK, 128], BF16, tag="tabsb")
                    nc.vector.tensor_copy(tabsb[:, :, 0:64], h_t[:, :, :])
                    nc.vector.tensor_copy(tabsb[:, :, 64:66], s_f[1][:, :, :])
                    stm_f = np_pool.tile([128, NBLK, 2], F32, tag="stm_f")
                    nc.vector.tensor_copy(stm_f[:, :, :], tabsb[:, :, 64:66])
                    nc.vector.tensor_tensor(tabsb[:, :, 66:68], s_f[1][:, :, :], stm_f[:, :, :],
                                            OP.subtract)
                    nc.vector.memset(tabsb[:, :, 68:69], 1.0)
                    nc.vector.memset(tabsb[:, :, 69:128], 0.0)
                    nc.sync.dma_start(tab_in.rearrange("(b p) c -> p b c", p=128),
                                      tabsb[:, :, :])
                    if do_node_ag:
                        nc.gpsimd.collective_compute(
                            "AllGather", OP.bypass,
                            replica_groups=[list(range(CORES))],
                            ins=[tab_in.opt()], outs=[table.opt()])
                    else:
                        nc.sync.dma_start(table[0:M_PER_CORE, :],
                                          tab_in[:, :])

                    # s_r table (once, both layers)
                    if l == 0:
                        relsb = np_pool.tile([128, 4, DIM], F32, tag="relsb")
                        nc.sync.dma_start(relsb[:, :, :],
                                          rel_in.ap().rearrange("(b p) f -> p b f", p=128))
                        relt = np_pool.tile([64, 4, 128], F32, tag="relt")
                        for b in range(4):
                            ps = np_psum.tile([64, 128], F32, tag="trh")
                            nc.tensor.transpose(ps[:, :], relsb[:, b, :], idf[:, :])
                            nc.scalar.activation(relt[:, b, :], ps[:, :], AF.Copy)
                        srsb = np_pool.tile([128, 4, 128], BF16, tag="srsb")
                        nc.vector.memset(srsb[:, :, :], 0.0)
                        sr_f = np_pool.tile([128, 4, 2 * L_LAYERS], F32, tag="sr_f")
                        for ll in range(L_LAYERS):
                            for b in range(4):
                                node_scores_block(np_pool, np_psum, relt[:, b, :],
                                                  Wht[:, 2 * L_LAYERS + ll, :],
                                                  attr[:, 2 * L_LAYERS + ll, :],
                                                  sr_f[:, b, 2 * ll:2 * ll + 2])
                        nc.vector.tensor_copy(srsb[:, :, 0:4], sr_f[:, :, :])
                        srm_f = np_pool.tile([128, 4, 4], F32, tag="srm_f")
                        nc.vector.tensor_copy(srm_f[:, :, :], srsb[:, :, 0:4])
                        nc.vector.tensor_tensor(srsb[:, :, 4:8], sr_f[:, :, :], srm_f[:, :, :],
                                                OP.subtract)
                        nc.sync.dma_start(srtab.rearrange("(b p) c -> p b c", p=128),
                                          srsb[:, :, :])

                # ================= edge phase =================
                for it in range(n_iters):
                    with tc.tile_pool(name="edgep", bufs=3) as ep, \
                         tc.tile_pool(name="edgeps", bufs=2) as eps, \
                         tc.tile_pool(name="edgep2", bufs=2) as ep2, \
                         tc.tile_pool(name="spmm_ps", bufs=2, space="PSUM") as spmm_ps, \
                         tc.tile_pool(name="zps_pool", bufs=2, space="PSUM") as zps_pool:
                        for k in range(NPIECE):
                            psb = spmm_ps.tile([128, BPP, 128], F32, tag="blkps")
                            psz = zps_pool.tile([128, BPP, 2], F32, tag="zps")
                            for p in range(2):
                                slot0 = p * C_PASS + k * PC
                                Gt = ep.tile([128, PC, 128], BF16, tag="Gt")
                                nc.gpsimd.dma_gather(
                                    out_ap=Gt[:, :, :],
                                    in_ap=table[p * SPLIT:, :],
                                    idxs_ap=gidx[:, p, k * (NP_IDX // 16):(k + 1) * (NP_IDX // 16)],
                                    num_idxs=NP_IDX, num_idxs_reg=NP_IDX, elem_size=128, single_packet=False)
                                # transposed one-hot [128e, 128n, c]; rebuilt on
                                # DVE every iteration (2x 16-bit mode via (j2,k)
                                # pair views) — cheaper than DRAM caching.
                                oh_t = ep2.tile([128, 128, PC], BF16, tag="oh")
                                nc.vector.tensor_tensor(
                                    oh_t.rearrange("p n (j2 e) -> p n j2 e", e=2),
                                    offs[:, slot0:slot0 + PC]
                                        .rearrange("p (j2 e) -> p j2 e", e=2)
                                        .unsqueeze(1)
                                        .broadcast_to([128, 128, PC // 2, 2]),
                                    iota.rearrange("p (n e) -> p n e", e=2)
                                        .unsqueeze(2)
                                        .broadcast_to([128, 128, PC // 2, 2]),
                                    OP.is_equal)

                                if it == 0:
                                    idx0 = (p * NPIECE + k) * (NP_IDX // 16)
                                    idx1 = (p * NPIECE + k + 1) * (NP_IDX // 16)
                                    if l == 0:
                                        SRt = eps.tile([128, PC, 128], BF16, tag="SRt")
                                        nc.gpsimd.dma_gather(
                                            out_ap=SRt[:, :, :],
                                            in_ap=srtab[:, :],
                                            idxs_ap=tidx[:, idx0:idx1],
                                            num_idxs=NP_IDX, num_idxs_reg=NP_IDX, elem_size=128, single_packet=False)
                                        nc.vector.tensor_copy(
                                            SRx[:, slot0:slot0 + PC, :], SRt[:, :, 0:8])
                                    SHt = eps.tile([128, PC, 128], BF16, tag="SHt")
                                    nc.gpsimd.dma_gather(
                                        out_ap=SHt[:, :, :],
                                        in_ap=shtab[:, :],
                                        idxs_ap=shidx[:, idx0:idx1],
                                        num_idxs=NP_IDX, num_idxs_reg=NP_IDX, elem_size=128, single_packet=False)
                                    # scores
                                    sc = ep2.tile([128, PC, 2], F32, tag="sc")
                                    nc.vector.tensor_tensor(sc[:, :, :], Gt[:, :, 64:66],
                                                            Gt[:, :, 66:68], OP.add)
                                    t2 = ep2.tile([128, PC, 2], F32, tag="t2")
                                    nc.vector.tensor_tensor(
                                        t2[:, :, :],
                                        SRx[:, slot0:slot0 + PC, 2 * l:2 * l + 2],
                                        SRx[:, slot0:slot0 + PC, 4 + 2 * l:6 + 2 * l], OP.add)
                                    nc.vector.tensor_tensor(sc[:, :, :], sc[:, :, :], t2[:, :, :],
                                                            OP.add)
                                    nc.vector.tensor_tensor(t2[:, :, :], SHt[:, :, 0:2],
                                                            SHt[:, :, 2:4], OP.add)
                                    nc.vector.tensor_tensor(sc[:, :, :], sc[:, :, :], t2[:, :, :],
                                                            OP.add)
                                    nc.vector.scalar_tensor_tensor(
                                        sc[:, :, :], sc[:, :, :], LRELU, sc[:, :, :],
                                        OP.mult, OP.max)
                                    nc.scalar.activation(At[:, slot0:slot0 + PC, :],
                                                         sc[:, :, :], AF.Exp)

                                # messages, interleaved cols (c = 2d+h).
                                A4 = At[:, slot0:slot0 + PC, :].unsqueeze(2) \
                                    .broadcast_to([128, PC, 64, 2])
                                if it == 0:
                                    # Z0 = h (cols 0:64) for both heads
                                    msg = ep.tile([128, PC, 128], BF16, tag="Gt")
                                    nc.vector.tensor_tensor(
                                        msg.rearrange("p c (d h) -> p c d h", h=2),
                                        Gt[:, :, 0:64].unsqueeze(3)
                                            .broadcast_to([128, PC, 64, 2]),
                                        A4, OP.mult)
                                else:
                                    # in place: same column elementwise (2x DVE)
                                    msg = Gt
                                    G4 = Gt.rearrange("p c (d h) -> p c d h", h=2)
                                    nc.vector.tensor_tensor(G4, G4, A4, OP.mult)
                                # spmm (+ z columns via separate 2-col matmul on iter 0)
                                # PSUM zero-regions are whole banks: start/stop once per bank.
                                bank1_j0 = meta["BANK1_J0"]
                                for j in range(PC):
                                    bl = J2BL[j]
                                    st = (p == 0) and (j == 0 or j == bank1_j0)
                                    sp = (p == 1) and (j == bank1_j0 - 1 or j == PC - 1)
                                    nc.tensor.matmul(
                                        psb[:, bl, :], oh_t[:, :, j], msg[:, j, :],
                                        start=st, stop=sp)
                                    if it == 0:
                                        nc.tensor.matmul(
                                            psz[:, bl, :], oh_t[:, :, j],
                                            At[:, slot0 + j, :],
                                            start=(p == 0 and j == 0),
                                            stop=(p == 1 and j == PC - 1))

                            # piece epilogue: z, recip, Z assembly
                            if it == 0:
                                b0 = k * BPP
                                nc.vector.tensor_scalar(zt[:, b0:b0 + BPP, :], psz[:, :, :],
                                                        1e-30, None, OP.max)
                                nc.vector.reciprocal(recip[:, b0:b0 + BPP, :],
                                                     zt[:, b0:b0 + BPP, :])
                                nc.vector.tensor_scalar(recip[:, b0:b0 + BPP, :],
                                                        recip[:, b0:b0 + BPP, :],
                                                        1.0 - ALPHA, None, OP.mult)
                            for bl in range(BPP):
                                b = k * BPP + bl
                                Z4b = Z[:, b, :].rearrange("p (d h) -> p d h", h=2)
                                psb4 = psb[:, bl, :].rearrange("p (d h) -> p d h", h=2)
                                nc.scalar.activation(Z4b[:, :, 0], psb4[:, :, 0], AF.Copy,
                                                     scale=recip[:, b, 0:1])
                                nc.scalar.activation(Z4b[:, :, 1], psb4[:, :, 1],
                                                     AF.Copy, scale=recip[:, b, 1:2])
                                nc.vector.scalar_tensor_tensor(
                                    Z4b,
                                    h_t[:, b, :].unsqueeze(2)
                                        .broadcast_to([128, 64, 2]),
                                    ALPHA, Z4b,
                                    OP.mult, OP.add)

                        if debug and l == 0 and it == 0:
                            nc.sync.dma_start(dbg_z0.ap().rearrange("(b p) c -> p b c", p=128),
                                              Z[:, :, :])
                            nc.sync.dma_start(
                                dbg_at.ap(),
                                At[:, :, :])

                        if it < n_iters - 1:
                            tabz = ep.tile([128, NBLK, 128], BF16, tag="Gt")
                            nc.vector.tensor_copy(tabz[:, :, :], Z[:, :, :])
                            nc.sync.dma_start(tab_in.rearrange("(b p) c -> p b c", p=128),
                                              tabz[:, :, :])
                            if do_ag:
                                nc.gpsimd.collective_compute(
                                    "AllGather", OP.bypass,
                                    replica_groups=[list(range(CORES))],
                                    ins=[tab_in.opt()], outs=[table.opt()])

                # ================= conv + residual =================
                with tc.tile_pool(name="convp", bufs=2) as cp, \
                     tc.tile_pool(name="convps", bufs=4, space="PSUM") as cps:
                    for b in range(NBLK):
                        pzt = cps.tile([128, 128], F32, tag="pzt")
                        nc.tensor.transpose(pzt[:, :], Z[:, b, :], idf[:, :])
                        Zt = cp.tile([128, 128], F32, tag="Zt")
                        nc.scalar.activation(Zt[:, :], pzt[:, :], AF.Copy)
                        pc_ = cps.tile([128, 64], F32, tag="pc")
                        nc.tensor.matmul(pc_[:, :], Zt[:, :], Wo[:, l, :],
                                         start=True, stop=True)
                        nc.vector.tensor_tensor(ent[:, b, :], pc_[:, :], ent[:, b, :], OP.add)

            nc.sync.dma_start(out_ext.ap().rearrange("(b p) f -> p b f", p=128),
                              ent[:, :, :])

    nc.compile()
    return nc


# ----------------------------------------------------------------------------
# host wrapper
# ----------------------------------------------------------------------------

def _make_in_maps(inputs, plans):
    ent = np.asarray(inputs["entity_embed"], np.float32)
    rel = np.zeros((512, DIM), np.float32)
    rel[:N_REL] = np.asarray(inputs["relation_embed"], np.float32)
    lng = np.tile(np.asarray(inputs["ln_gamma"], np.float32)[None], (128, 1, 1))
    lnb = np.tile(np.asarray(inputs["ln_beta"], np.float32)[None], (128, 1, 1))
    W = np.stack([np.asarray(inputs["W_h"], np.float32),
                  np.asarray(inputs["W_t"], np.float32),
                  np.asarray(inputs["W_r"], np.float32)], axis=0)
    att = np.stack([np.asarray(inputs["att_h"], np.float32),
                    np.asarray(inputs["att_t"], np.float32),
                    np.asarray(inputs["att_r"], np.float32)], axis=0)
    att_rep = np.tile(att.reshape(3, L_LAYERS, 1, HD), (1, 1, 128, 1)).astype(np.float32)
    # Z columns are interleaved as c = 2d+h on device; permute W_o rows to match
    cperm = (np.arange(128) % 2) * 64 + np.arange(128) // 2
    Wo = np.ascontiguousarray(np.asarray(inputs["W_o"], np.float32)[:, cperm, :])
    iota = np.tile(np.repeat(np.arange(128, dtype=np.float32), 2)[None],
                   (128, 1)).astype(BF)
    idf = np.eye(128, dtype=np.float32)

    common = dict(rel_emb=rel, ln_g=np.ascontiguousarray(lng), ln_b=np.ascontiguousarray(lnb),
                  W_htr=np.ascontiguousarray(W), att_rep=np.ascontiguousarray(att_rep),
                  W_o=np.ascontiguousarray(Wo), iota=iota, ident_f=idf)
    in_maps = []
    for pl in plans:
        sl = np.zeros((M_PER_CORE, DIM), np.float32)
        nv = pl["n_valid"]
        idx = pl["perm"]
        valid = idx < nv
        sl[valid] = ent[pl["node_lo"] + idx[valid]]
        m = dict(common)
        m["ent_slice"] = sl
        m["gidx"] = np.ascontiguousarray(np.stack(pl["gidx"], axis=1))
        m["tidx"] = pl["tidx"]
        m["shidx"] = pl["shidx"]
        m["offs"] = pl["offs"]
        in_maps.append(m)
    return in_maps


_CACHE = {}


def _get_nc(meta, debug):
    key = (meta["PIECE_CHUNKS"], meta["J2BL"], debug)
    if key not in _CACHE:
        _CACHE[key] = _build_nc(meta, debug=debug)
    return _CACHE[key]


def run(inputs, debug=False, trace=False):
    plans, meta = _build_plan(np.asarray(inputs["edge_index"]),
                              np.asarray(inputs["edge_type"]))
    nc = _get_nc(meta, debug)
    in_maps = _make_in_maps(inputs, plans)
    res = bass_utils.run_bass_kernel_spmd(nc, in_maps, core_ids=list(range(CORES)),
                                          trace=trace)
    out = np.zeros((N_ENT, DIM), np.float32)
    for c, pl in enumerate(plans):
        nv = pl["n_valid"]
        sl = np.asarray(res.results[c]["out_slice"])
        idx = pl["perm"]
        valid = idx < nv
        out[pl["node_lo"] + idx[valid]] = sl[valid]
    return out, res, plans, meta


def kernel(**inputs) -> np.ndarray:
    out, _, _, _ = run(inputs)
    return out.astype(np.asarray(inputs["entity_embed"]).dtype)



# revision 25
# speedup vs baseline: 1.3529x; 1.0815x over previous
"""Trainium2 Bass kernel for nn_DAGNLinkPrediction (GNN message passing).

Self-contained: host-side integer preprocessing (sharding/permutation) + bass/tile
kernel + SPMD launch across 8 NeuronCores via run_bass_kernel_spmd.

Sharding: edges partitioned by src range (6272 nodes/core). Per core, edges are
sorted by (dst>=SPLIT, src-block, src) and padded into 128-edge chunks confined
to 128-node blocks. Per power iteration each core gathers Z[dst] rows (256B bf16)
with gpsimd.dma_gather, computes messages, segment-sums by src via PE matmuls
with 0/1 one-hot matrices, and AllGathers the updated bf16 node table.
"""
import numpy as np
import ml_dtypes

from concourse import bass, bacc, tile, bass_utils, mybir

BF = ml_dtypes.bfloat16
F32 = mybir.dt.float32
BF16 = mybir.dt.bfloat16
I16 = mybir.dt.int16
I32 = mybir.dt.int32

CORES = 8
N_ENT = 50000
N_REL = 500
HEADS = 2
DIM = 64
HD = HEADS * DIM                 # 128
M_PER_CORE = 6272                # 49*128
NBLK = M_PER_CORE // 128         # 49
TAB_ROWS = CORES * M_PER_CORE    # 50176
SPLIT = TAB_ROWS // 2            # 25088
NPIECE = 7                       # pieces per pass; NBLK = 7*7
BPP = NBLK // NPIECE             # blocks per piece = 7
ALPHA = 0.15
LN_EPS = 1e-5
L_LAYERS = 2
POW_ITER = 3
LRELU = 0.01
N_EDGE_KEY = 8192                # > M_PER_CORE; packs (key, src pos) for sort

AF = mybir.ActivationFunctionType
OP = mybir.AluOpType


# ----------------------------------------------------------------------------
# host-side preprocessing (integer/layout only)
# ----------------------------------------------------------------------------

def _wrap_idxs(idx):
    n = idx.shape[0]
    base = idx.reshape(n // 16, 16).T.astype(np.int16)
    return np.ascontiguousarray(np.tile(base, (8, 1)))


def _pack_core(deg, caps):
    """Assign 6272 nodes (deg: [6272, 2] edge counts per dst-half) to 49
    blocks of exactly 128 nodes with per-block per-half edge capacity caps[b].
    Returns blk_of [6272] or None if infeasible."""
    n_nodes = deg.shape[0]
    order = np.argsort(-(deg.sum(1) * 64 + deg.max(1)), kind="stable")
    slots = np.full(NBLK, 128, np.int64)
    load = np.zeros((NBLK, 2), np.int64)
    blk_of = np.empty(n_nodes, np.int64)
    capsf = caps.astype(np.float64)
    for n in order:
        d0, d1 = deg[n]
        sc = np.maximum(load[:, 0] + d0, load[:, 1] + d1) / capsf
        sc[slots <= 0] = np.inf
        b = int(np.argmin(sc))
        if sc[b] > 1.0:
            return None
        blk_of[n] = b
        slots[b] -= 1
        load[b] += deg[n]
    return blk_of


def _build_plan(edge_index, edge_type):
    src = edge_index[0].astype(np.int64)
    dst = edge_index[1].astype(np.int64)
    et = edge_type.astype(np.int64)

    src_core = src // M_PER_CORE
    src_loc = src % M_PER_CORE
    hi = (dst >= SPLIT).astype(np.int64)

    # per-core, per-half node degrees
    deg = np.zeros((CORES, M_PER_CORE, 2), np.int64)
    np.add.at(deg, (src_core, src_loc, hi), 1)

    # find the smallest even per-piece chunk count with a feasible packing
    blk_of_all = None
    for PCP in range(36, 44, 2):
        prof_piece = [5] * NPIECE
        for i in range(PCP - 5 * NPIECE):
            prof_piece[i % NPIECE] += 1
        prof = np.array(prof_piece * NPIECE, np.int64)  # [NBLK]
        caps = prof * 128
        blk_of_all = []
        ok = True
        for c in range(CORES):
            blk_of = _pack_core(deg[c], caps)
            if blk_of is None:
                ok = False
                break
            blk_of_all.append(blk_of)
        if ok:
            break
    assert blk_of_all is not None and len(blk_of_all) == CORES, "packing failed"

    NCH = int(prof.sum())                     # chunks per pass
    NIDX = NCH * 128
    chunk0 = np.concatenate([[0], np.cumsum(prof)])  # chunk offset per block

    # new position of every node: newpos = blk*128 + slot (stable within block)
    newpos = np.empty((CORES, M_PER_CORE), np.int64)
    perms = []
    for c in range(CORES):
        order_nodes = np.argsort(blk_of_all[c] * M_PER_CORE
                                 + np.arange(M_PER_CORE), kind="stable")
        # order_nodes[i] = old local id placed at new position i
        perms.append(order_nodes)
        inv = np.empty(M_PER_CORE, np.int64)
        inv[order_nodes] = np.arange(M_PER_CORE)
        newpos[c] = inv
    dst_new = (dst // M_PER_CORE) * M_PER_CORE + newpos[dst // M_PER_CORE,
                                                        dst % M_PER_CORE]
    src_new = src_core * M_PER_CORE + newpos[src_core, src_loc]
    src_nloc = src_new % M_PER_CORE
    blk = src_nloc // 128

    key = (src_core * 2 + hi) * NBLK + blk
    sortkey = key * N_EDGE_KEY + src_nloc
    order = np.argsort(sortkey, kind="stable")
    bounds = np.searchsorted(sortkey[order],
                             np.arange(CORES * 2 * NBLK + 1) * N_EDGE_KEY,
                             side="left")

    plans = []
    for c in range(CORES):
        pl = {}
        pl["node_lo"] = c * M_PER_CORE
        pl["n_valid"] = max(0, min(N_ENT - c * M_PER_CORE, M_PER_CORE))
        pl["perm"] = perms[c]
        gidx, offs, typs, shl = [], [], [], []
        for p in range(2):
            eids = np.full(NIDX, -1, np.int64)
            for b in range(NBLK):
                k = (c * 2 + p) * NBLK + b
                lst = order[bounds[k]:bounds[k + 1]]
                s0 = chunk0[b] * 128
                assert len(lst) <= (chunk0[b + 1] - chunk0[b]) * 128
                eids[s0:s0 + len(lst)] = lst
            pad = eids < 0
            e_safe = np.where(pad, 0, eids)
            didx = dst_new[e_safe] - p * SPLIT
            didx[pad] = 0
            off = (src_nloc[e_safe] % 128)
            off[pad] = -1
            t = et[e_safe].copy()
            t[pad] = 0
            sl = src_nloc[e_safe].copy()
            sl[pad] = 0
            gidx.append(_wrap_idxs(didx.astype(np.int16)))
            offs.append(off.reshape(NCH, 128).T.astype(np.float32))  # [128, NCH]
            typs.append(t)
            shl.append(sl)
        pl["gidx"] = gidx
        pl["tidx"] = _wrap_idxs(np.concatenate(typs).astype(np.int16))
        pl["shidx"] = _wrap_idxs(np.concatenate(shl).astype(np.int16))
        pl["offs"] = np.concatenate(offs, axis=1).astype(BF)  # [128, 2*NCH]
        plans.append(pl)

    j2bl = []
    for b, cnt in enumerate(prof_piece):
        j2bl += [b] * cnt
    bank1_j0 = next(j for j, b in enumerate(j2bl) if b >= 4)
    meta = dict(C_PASS=NCH, NIDX=NIDX, PIECE_CHUNKS=PCP,
                NIDX_PIECE=PCP * 128, J2BL=tuple(j2bl), BANK1_J0=bank1_j0)
    return plans, meta


# ----------------------------------------------------------------------------
# device kernel
# ----------------------------------------------------------------------------

def _build_nc(meta, debug=False, n_layers=L_LAYERS, n_iters=POW_ITER, do_ag=True, do_node_ag=True, ln_trivial=False):
    C_PASS = meta["C_PASS"]
    J2BL = meta["J2BL"]
    PC = meta["PIECE_CHUNKS"]          # chunks per piece
    NP_IDX = meta["NIDX_PIECE"]        # idxs per piece
    IW = meta["NIDX"] // 16            # idx cols per pass

    nc = bacc.Bacc("TRN2", target_bir_lowering=False, debug=False,
                   num_devices=CORES)

    # ---- external inputs ----
    ent_in = nc.dram_tensor("ent_slice", [M_PER_CORE, DIM], F32, kind="ExternalInput")
    rel_in = nc.dram_tensor("rel_emb", [512, DIM], F32, kind="ExternalInput")
    lng_in = nc.dram_tensor("ln_g", [128, L_LAYERS, DIM], F32, kind="ExternalInput")
    lnb_in = nc.dram_tensor("ln_b", [128, L_LAYERS, DIM], F32, kind="ExternalInput")
    W_in = nc.dram_tensor("W_htr", [3, L_LAYERS, DIM, HD], F32, kind="ExternalInput")
    att_in = nc.dram_tensor("att_rep", [3, L_LAYERS, 128, HD], F32, kind="ExternalInput")
    Wo_in = nc.dram_tensor("W_o", [L_LAYERS, HD, DIM], F32, kind="ExternalInput")
    gidx_in = nc.dram_tensor("gidx", [128, 2, IW], I16, kind="ExternalInput")
    tidx_in = nc.dram_tensor("tidx", [128, 2 * IW], I16, kind="ExternalInput")
    shidx_in = nc.dram_tensor("shidx", [128, 2 * IW], I16, kind="ExternalInput")
    offs_in = nc.dram_tensor("offs", [128, 2 * C_PASS], BF16, kind="ExternalInput")
    iota_in = nc.dram_tensor("iota", [128, 256], BF16, kind="ExternalInput")
    magic_in = nc.dram_tensor("rsqrt_magic", [128, 1], I32, kind="ExternalInput")
    idf_in = nc.dram_tensor("ident_f", [128, 128], F32, kind="ExternalInput")

    out_ext = nc.dram_tensor("out_slice", [M_PER_CORE, DIM], F32, kind="ExternalOutput")
    if debug:
        dbg_at = nc.dram_tensor("dbg_at", [128, 2 * C_PASS, 2], BF16, kind="ExternalOutput")
        dbg_z0 = nc.dram_tensor("dbg_z0", [M_PER_CORE, HD], F32, kind="ExternalOutput")
        dbg_h = nc.dram_tensor("dbg_h", [M_PER_CORE, DIM], F32, kind="ExternalOutput")

    with tile.TileContext(nc) as tc:
        with tc.tile_pool(name="dram", bufs=1, space="DRAM") as dram, \
             tc.tile_pool(name="persist", bufs=1) as pp:
            table = dram.tile([TAB_ROWS, 128], BF16, tag="table")
            tab_in = dram.tile([M_PER_CORE, 128], BF16, tag="tab_in")
            srtab = dram.tile([512, 128], BF16, tag="srtab")
            shtab = dram.tile([M_PER_CORE, 128], BF16, tag="shtab")

            ent = pp.tile([128, NBLK, DIM], F32, tag="ent")
            h_t = pp.tile([128, NBLK, DIM], F32, tag="h")
            recip = pp.tile([128, NBLK, 2], F32, tag="recip")
            zt = pp.tile([128, NBLK, 2], F32, tag="zt")
            At = pp.tile([128, 2 * C_PASS, 2], BF16, tag="At")
            SRx = pp.tile([128, 2 * C_PASS, 8], BF16, tag="SRx")
            Z = pp.tile([128, NBLK, HD], F32, tag="Z")
            gidx = pp.tile([128, 2, IW], I16, tag="gidx")
            tidx = pp.tile([128, 2 * IW], I16, tag="tidx")
            shidx = pp.tile([128, 2 * IW], I16, tag="shidx")
            offs = pp.tile([128, 2 * C_PASS], BF16, tag="offs")
            iota = pp.tile([128, 256], BF16, tag="iota")
            magici = pp.tile([128, 1], I32, tag="magic")
            idf = pp.tile([128, 128], F32, tag="idf")
            lng = pp.tile([128, L_LAYERS, DIM], F32, tag="lng")
            lnb = pp.tile([128, L_LAYERS, DIM], F32, tag="lnb")
            Wht = pp.tile([64, 3 * L_LAYERS, HD], F32, tag="Wht")
            attr = pp.tile([128, 3 * L_LAYERS, HD], F32, tag="attr")
            Wo = pp.tile([HD, L_LAYERS, DIM], F32, tag="Wo")

            # ---- load inputs ----
            nc.sync.dma_start(ent[:, :, :], ent_in.ap().rearrange("(b p) f -> p b f", p=128))
            nc.sync.dma_start(gidx[:, :, :], gidx_in.ap())
            nc.sync.dma_start(tidx[:, :], tidx_in.ap())
            nc.sync.dma_start(shidx[:, :], shidx_in.ap())
            nc.sync.dma_start(offs[:, :], offs_in.ap())
            nc.sync.dma_start(iota[:, :], iota_in.ap())
            nc.sync.dma_start(magici[:, :], magic_in.ap())
            nc.sync.dma_start(idf[:, :], idf_in.ap())
            nc.sync.dma_start(lng[:, :, :], lng_in.ap())
            nc.sync.dma_start(lnb[:, :, :], lnb_in.ap())
            nc.sync.dma_start(Wht[:, :, :], W_in.ap().rearrange("r l k m -> k (r l) m"))
            nc.sync.dma_start(attr[:, :, :], att_in.ap().rearrange("r l p m -> p (r l) m"))
            nc.sync.dma_start(Wo[:, :, :], Wo_in.ap().rearrange("l k m -> k l m"))

            def node_scores_block(pool, psum, lhsT, Wslice, att_ap, out_ap,
                                  eng=None):
                """tanh(x@W) . att summed over d -> out_ap [128,2] (f32)."""
                eng = eng or nc.vector
                pt = psum.tile([128, HD], F32, tag="nps")
                nc.tensor.matmul(pt[:, :], lhsT, Wslice, start=True, stop=True)
                tt = pool.tile([128, HD], F32, tag="ns_tt")
                nc.scalar.activation(tt[:, :], pt[:, :], AF.Tanh)
                tm = pool.tile([128, HD], F32, tag="ns_tm")
                eng.tensor_tensor(tm[:, :], tt[:, :], att_ap, OP.mult)
                nc.vector.tensor_reduce(out_ap, tm.rearrange("p (h d) -> p h d", h=2),
                                        mybir.AxisListType.X, OP.add)

            # persistent per-node score tensors (written per piece)
            s_f0 = pp.tile([128, NBLK, 2], F32, tag="s_f0")
            s_f1 = pp.tile([128, NBLK, 2], F32, tag="s_f1")
            s_f = [s_f0, s_f1]

            def node_piece(np_pool, np_psum, l, kb):
                """LN + node scores + shtab/tab_in rows for piece kb's blocks."""
                b0 = kb * BPP
                sl = slice(b0, b0 + BPP)
                x = ent[:, sl, :]
                mu = np_pool.tile([128, BPP], F32, tag="mu")
                nc.vector.tensor_reduce(mu[:, :], x, mybir.AxisListType.X, OP.add)
                nc.vector.tensor_scalar(mu[:, :], mu[:, :], -1.0 / DIM, None, OP.mult)
                xc = np_pool.tile([128, BPP, DIM], F32, tag="xc")
                nc.vector.tensor_tensor(
                    xc[:, :, :], x,
                    mu.unsqueeze(2).broadcast_to([128, BPP, DIM]), OP.add)
                sq = np_pool.tile([128, BPP, DIM], F32, tag="sq")
                nc.scalar.activation(sq[:, :, :], xc[:, :, :], AF.Square)
                var = np_pool.tile([128, BPP], F32, tag="var")
                nc.vector.tensor_reduce(var[:, :], sq[:, :, :], mybir.AxisListType.X, OP.add)
                nc.vector.tensor_scalar(var[:, :], var[:, :], 1.0 / DIM, LN_EPS, OP.mult, OP.add)
                rstd = np_pool.tile([128, BPP], F32, tag="rstd")
                yi = np_pool.tile([128, BPP], I32, tag="yi")
                nc.vector.tensor_scalar(yi[:, :], var[:, :].bitcast(I32), 1, None,
                                        OP.logical_shift_right)
                nc.vector.tensor_tensor(
                    yi[:, :],
                    magici[:, 0].unsqueeze(1).broadcast_to([128, BPP]),
                    yi[:, :], OP.subtract)
                yf = yi[:, :].bitcast(F32)
                nrt = np_pool.tile([128, BPP], F32, tag="nrt")
                nc.vector.tensor_tensor(nrt[:, :], yf, yf, OP.mult)
                nc.vector.tensor_tensor(nrt[:, :], nrt[:, :], var[:, :], OP.mult)
                nc.vector.tensor_scalar(nrt[:, :], nrt[:, :], -0.5, 1.5, OP.mult, OP.add)
                nc.vector.tensor_tensor(rstd[:, :], yf, nrt[:, :], OP.mult)
                nc.vector.tensor_tensor(nrt[:, :], rstd[:, :], rstd[:, :], OP.mult)
                nc.vector.tensor_tensor(nrt[:, :], nrt[:, :], var[:, :], OP.mult)
                nc.vector.tensor_scalar(nrt[:, :], nrt[:, :], -0.5, 1.5, OP.mult, OP.add)
                nc.vector.tensor_tensor(rstd[:, :], rstd[:, :], nrt[:, :], OP.mult)
                nc.vector.tensor_tensor(
                    h_t[:, sl, :], xc[:, :, :],
                    rstd.unsqueeze(2).broadcast_to([128, BPP, DIM]), OP.mult)
                if not ln_trivial:
                    nc.vector.tensor_tensor(
                        h_t[:, sl, :], h_t[:, sl, :],
                        lng[:, l, :].unsqueeze(1).broadcast_to([128, BPP, DIM]),
                        OP.mult)
                    nc.vector.tensor_tensor(
                        h_t[:, sl, :], h_t[:, sl, :],
                        lnb[:, l, :].unsqueeze(1).broadcast_to([128, BPP, DIM]),
                        OP.add)
                ht = np_pool.tile([64, BPP, 128], F32, tag="ht")
                for bl in range(BPP):
                    ps = np_psum.tile([64, 128], F32, tag="nps")
                    nc.tensor.transpose(ps[:, :], h_t[:, b0 + bl, :], idf[:, :])
                    nc.scalar.activation(ht[:, bl, :], ps[:, :], AF.Copy)
                for r in range(2):
                    for bl in range(BPP):
                        node_scores_block(np_pool, np_psum, ht[:, bl, :],
                                          Wht[:, r * L_LAYERS + l, :],
                                          attr[:, r * L_LAYERS + l, :],
                                          s_f[r][:, b0 + bl, :],
                                          eng=nc.gpsimd)
                # sh table rows: [bf16(s_h)(2), bf16(residual)(2), 0...]
                shsb = np_pool.tile([128, BPP, 128], BF16, tag="shsb")
                shm_f = np_pool.tile([128, BPP, 2], F32, tag="shm_f")
                nc.vector.tensor_copy(shsb[:, :, 0:2], s_f[0][:, sl, :])
                nc.vector.tensor_copy(shm_f[:, :, :], shsb[:, :, 0:2])
                nc.vector.tensor_tensor(shsb[:, :, 2:4], s_f[0][:, sl, :],
                                        shm_f[:, :, :], OP.subtract)
                nc.sync.dma_start(
                    shtab.rearrange("(b p) c -> p b c", p=128)[:, sl, :],
                    shsb[:, :, :])
                # gather table rows: [h | s_t | s_t_res | 0...]
                tabsb = np_pool.tile([128, BPP, 128], BF16, tag="tabsb")
                nc.vector.tensor_copy(tabsb[:, :, 0:64], h_t[:, sl, :])
                nc.vector.tensor_copy(tabsb[:, :, 64:66], s_f[1][:, sl, :])
                stm_f = np_pool.tile([128, BPP, 2], F32, tag="stm_f")
                nc.vector.tensor_copy(stm_f[:, :, :], tabsb[:, :, 64:66])
                nc.vector.tensor_tensor(tabsb[:, :, 66:68], s_f[1][:, sl, :],
                                        stm_f[:, :, :], OP.subtract)
                nc.sync.dma_start(
                    tab_in.rearrange("(b p) c -> p b c", p=128)[:, sl, :],
                    tabsb[:, :, :])

            def conv_piece(cp, cps, l, kb):
                """ent[blocks of kb] += Z @ W_o (residual update)."""
                for bl in range(BPP):
                    b = kb * BPP + bl
                    pzt = cps.tile([128, 128], F32, tag="cvps")
                    nc.tensor.transpose(pzt[:, :], Z[:, b, :], idf[:, :])
                    Zt = cp.tile([128, 128], F32, tag="Zt")
                    nc.scalar.activation(Zt[:, :], pzt[:, :], AF.Copy)
                    pc_ = cps.tile([128, 64], F32, tag="cvps")
                    nc.tensor.matmul(pc_[:, :], Zt[:, :], Wo[:, l, :],
                                     start=True, stop=True)
                    nc.vector.tensor_tensor(ent[:, b, :], pc_[:, :], ent[:, b, :],
                                            OP.add)

            def node_ag():
                if do_node_ag:
                    nc.gpsimd.collective_compute(
                        "AllGather", OP.bypass,
                        replica_groups=[list(range(CORES))],
                        ins=[tab_in.opt()], outs=[table.opt()])
                else:
                    nc.sync.dma_start(table[0:M_PER_CORE, :], tab_in[:, :])

            # ---- prologue: layer-0 node phase + relation score table ----
            with tc.tile_pool(name="nodep", bufs=2) as np_pool, \
                 tc.tile_pool(name="nodeps", bufs=2, space="PSUM") as np_psum:
                for kb in range(NPIECE):
                    node_piece(np_pool, np_psum, 0, kb)
                node_ag()
                if debug:
                    nc.sync.dma_start(dbg_h.ap().rearrange("(b p) f -> p b f", p=128),
                                      h_t[:, :, :])
                # s_r table (once, both layers)
                relsb = np_pool.tile([128, 4, DIM], F32, tag="relsb")
                nc.sync.dma_start(relsb[:, :, :],
                                  rel_in.ap().rearrange("(b p) f -> p b f", p=128))
                relt = np_pool.tile([64, 4, 128], F32, tag="relt")
                for b in range(4):
                    ps = np_psum.tile([64, 128], F32, tag="nps")
                    nc.tensor.transpose(ps[:, :], relsb[:, b, :], idf[:, :])
                    nc.scalar.activation(relt[:, b, :], ps[:, :], AF.Copy)
                srsb = np_pool.tile([128, 4, 128], BF16, tag="srsb")
                nc.vector.memset(srsb[:, :, :], 0.0)
                sr_f = np_pool.tile([128, 4, 2 * L_LAYERS], F32, tag="sr_f")
                for ll in range(L_LAYERS):
                    for b in range(4):
                        node_scores_block(np_pool, np_psum, relt[:, b, :],
                                          Wht[:, 2 * L_LAYERS + ll, :],
                                          attr[:, 2 * L_LAYERS + ll, :],
                                          sr_f[:, b, 2 * ll:2 * ll + 2])
                nc.vector.tensor_copy(srsb[:, :, 0:4], sr_f[:, :, :])
                srm_f = np_pool.tile([128, 4, 4], F32, tag="srm_f")
                nc.vector.tensor_copy(srm_f[:, :, :], srsb[:, :, 0:4])
                nc.vector.tensor_tensor(srsb[:, :, 4:8], sr_f[:, :, :], srm_f[:, :, :],
                                        OP.subtract)
                nc.sync.dma_start(srtab.rearrange("(b p) c -> p b c", p=128),
                                  srsb[:, :, :])

            for l in range(n_layers):
                # ================= edge phase =================
                for it in range(n_iters):
                    last = it == n_iters - 1
                    with tc.tile_pool(name="edgep", bufs=3) as ep, \
                         tc.tile_pool(name="edgeps", bufs=2) as eps, \
                         tc.tile_pool(name="edgep2", bufs=2) as ep2, \
                         tc.tile_pool(name="spmm_ps", bufs=2, space="PSUM") as spmm_ps, \
                         tc.tile_pool(name="auxps", bufs=1, space="PSUM") as auxps, \
                         tc.tile_pool(name="nodep", bufs=2) as np_pool, \
                         tc.tile_pool(name="convp", bufs=2) as cp, \
                         tc.tile_pool(name="nodeps", bufs=2, space="PSUM") as np_psum:
                        for k in range(NPIECE):
                            psb = spmm_ps.tile([128, BPP, 128], F32, tag="blkps")
                            psz = auxps.tile([128, BPP, 2], F32, tag="zps")
                            for p in range(2):
                                slot0 = p * C_PASS + k * PC
                                Gt = ep.tile([128, PC, 128], BF16, tag="Gt")
                                nc.gpsimd.dma_gather(
                                    out_ap=Gt[:, :, :],
                                    in_ap=table[p * SPLIT:, :],
                                    idxs_ap=gidx[:, p, k * (NP_IDX // 16):(k + 1) * (NP_IDX // 16)],
                                    num_idxs=NP_IDX, num_idxs_reg=NP_IDX, elem_size=128, single_packet=False)
                                # transposed one-hot [128e, 128n, c]; rebuilt on
                                # DVE every iteration (2x 16-bit mode via (j2,k)
                                # pair views) — cheaper than DRAM caching.
                                oh_t = ep2.tile([128, 128, PC], BF16, tag="oh")
                                nc.vector.tensor_tensor(
                                    oh_t.rearrange("p n (j2 e) -> p n j2 e", e=2),
                                    offs[:, slot0:slot0 + PC]
                                        .rearrange("p (j2 e) -> p j2 e", e=2)
                                        .unsqueeze(1)
                                        .broadcast_to([128, 128, PC // 2, 2]),
                                    iota.rearrange("p (n e) -> p n e", e=2)
                                        .unsqueeze(2)
                                        .broadcast_to([128, 128, PC // 2, 2]),
                                    OP.is_equal)

                                if it == 0:
                                    idx0 = (p * NPIECE + k) * (NP_IDX // 16)
                                    idx1 = (p * NPIECE + k + 1) * (NP_IDX // 16)
                                    if l == 0:
                                        SRt = eps.tile([128, PC, 128], BF16, tag="SRt")
                                        nc.gpsimd.dma_gather(
                                            out_ap=SRt[:, :, :],
                                            in_ap=srtab[:, :],
                                            idxs_ap=tidx[:, idx0:idx1],
                                            num_idxs=NP_IDX, num_idxs_reg=NP_IDX, elem_size=128, single_packet=False)
                                        nc.vector.tensor_copy(
                                            SRx[:, slot0:slot0 + PC, :], SRt[:, :, 0:8])
                                    SHt = eps.tile([128, PC, 128], BF16, tag="SHt")
                                    nc.gpsimd.dma_gather(
                                        out_ap=SHt[:, :, :],
                                        in_ap=shtab[:, :],
                                        idxs_ap=shidx[:, idx0:idx1],
                                        num_idxs=NP_IDX, num_idxs_reg=NP_IDX, elem_size=128, single_packet=False)
                                    # scores
                                    sc = ep2.tile([128, PC, 2], F32, tag="sc")
                                    nc.vector.tensor_tensor(sc[:, :, :], Gt[:, :, 64:66],
                                                            Gt[:, :, 66:68], OP.add)
                                    t2 = ep2.tile([128, PC, 2], F32, tag="t2")
                                    nc.vector.tensor_tensor(
                                        t2[:, :, :],
                                        SRx[:, slot0:slot0 + PC, 2 * l:2 * l + 2],
                                        SRx[:, slot0:slot0 + PC, 4 + 2 * l:6 + 2 * l], OP.add)
                                    nc.vector.tensor_tensor(sc[:, :, :], sc[:, :, :], t2[:, :, :],
                                                            OP.add)
                                    nc.vector.tensor_tensor(t2[:, :, :], SHt[:, :, 0:2],
                                                            SHt[:, :, 2:4], OP.add)
                                    nc.vector.tensor_tensor(sc[:, :, :], sc[:, :, :], t2[:, :, :],
                                                            OP.add)
                                    nc.vector.scalar_tensor_tensor(
                                        sc[:, :, :], sc[:, :, :], LRELU, sc[:, :, :],
                                        OP.mult, OP.max)
                                    nc.scalar.activation(At[:, slot0:slot0 + PC, :],
                                                         sc[:, :, :], AF.Exp)

                                # messages, interleaved cols (c = 2d+h).
                                A4 = At[:, slot0:slot0 + PC, :].unsqueeze(2) \
                                    .broadcast_to([128, PC, 64, 2])
                                if it == 0:
                                    # Z0 = h (cols 0:64) for both heads
                                    msg = ep.tile([128, PC, 128], BF16, tag="Gt")
                                    nc.vector.tensor_tensor(
                                        msg.rearrange("p c (d h) -> p c d h", h=2),
                                        Gt[:, :, 0:64].unsqueeze(3)
                                            .broadcast_to([128, PC, 64, 2]),
                                        A4, OP.mult)
                                else:
                                    # in place: same column elementwise (2x DVE)
                                    msg = Gt
                                    G4 = Gt.rearrange("p c (d h) -> p c d h", h=2)
                                    nc.vector.tensor_tensor(G4, G4, A4, OP.mult)
                                # spmm (+ z columns via separate 2-col matmul on iter 0)
                                # PSUM zero-regions are whole banks: start/stop once per bank.
                                bank1_j0 = meta["BANK1_J0"]
                                for j in range(PC):
                                    bl = J2BL[j]
                                    st = (p == 0) and (j == 0 or j == bank1_j0)
                                    sp = (p == 1) and (j == bank1_j0 - 1 or j == PC - 1)
                                    nc.tensor.matmul(
                                        psb[:, bl, :], oh_t[:, :, j], msg[:, j, :],
                                        start=st, stop=sp)
                                    if it == 0:
                                        nc.tensor.matmul(
                                            psz[:, bl, :], oh_t[:, :, j],
                                            At[:, slot0 + j, :],
                                            start=(p == 0 and j == 0),
                                            stop=(p == 1 and j == PC - 1))

                            # piece epilogue: z, recip, Z assembly
                            if it == 0:
                                b0 = k * BPP
                                nc.vector.tensor_scalar(zt[:, b0:b0 + BPP, :], psz[:, :, :],
                                                        1e-30, None, OP.max)
                                nc.vector.reciprocal(recip[:, b0:b0 + BPP, :],
                                                     zt[:, b0:b0 + BPP, :])
                                nc.vector.tensor_scalar(recip[:, b0:b0 + BPP, :],
                                                        recip[:, b0:b0 + BPP, :],
                                                        1.0 - ALPHA, None, OP.mult)
                            for bl in range(BPP):
                                b = k * BPP + bl
                                Z4b = Z[:, b, :].rearrange("p (d h) -> p d h", h=2)
                                psb4 = psb[:, bl, :].rearrange("p (d h) -> p d h", h=2)
                                nc.scalar.activation(Z4b[:, :, 0], psb4[:, :, 0], AF.Copy,
                                                     scale=recip[:, b, 0:1])
                                nc.scalar.activation(Z4b[:, :, 1], psb4[:, :, 1],
                                                     AF.Copy, scale=recip[:, b, 1:2])
                                nc.vector.scalar_tensor_tensor(
                                    Z4b,
                                    h_t[:, b, :].unsqueeze(2)
                                        .broadcast_to([128, 64, 2]),
                                    ALPHA, Z4b,
                                    OP.mult, OP.add)
                            if not last:
                                # stream this piece's Z rows out now (casting
                                # f32->bf16 in the DMA) so the AllGather isn't
                                # gated on one big end-of-iteration write
                                b0 = k * BPP
                                nc.gpsimd.dma_start(
                                    tab_in.rearrange("(b p) c -> p b c", p=128)
                                        [:, b0:b0 + BPP, :],
                                    Z[:, b0:b0 + BPP, :])
                            else:
                                # fold conv+residual and the next layer's node
                                # phase for this piece under the iteration
                                conv_piece(cp, auxps, l, k)
                                if l + 1 < n_layers:
                                    node_piece(np_pool, np_psum, l + 1, k)

                        if debug and l == 0 and it == 0:
                            nc.sync.dma_start(dbg_z0.ap().rearrange("(b p) c -> p b c", p=128),
                                              Z[:, :, :])
                            nc.sync.dma_start(
                                dbg_at.ap(),
                                At[:, :, :])

                        if not last:
                            if do_ag:
                                nc.gpsimd.collective_compute(
                                    "AllGather", OP.bypass,
                                    replica_groups=[list(range(CORES))],
                                    ins=[tab_in.opt()], outs=[table.opt()])

                if l + 1 < n_layers:
                    node_ag()

            nc.sync.dma_start(out_ext.ap().rearrange("(b p) f -> p b f", p=128),
                              ent[:, :, :])

    nc.compile()
    return nc


# ----------------------------------------------------------------------------
# host wrapper
# ----------------------------------------------------------------------------

def _make_in_maps(inputs, plans):
    ent = np.asarray(inputs["entity_embed"], np.float32)
    rel = np.zeros((512, DIM), np.float32)
    rel[:N_REL] = np.asarray(inputs["relation_embed"], np.float32)
    lng = np.tile(np.asarray(inputs["ln_gamma"], np.float32)[None], (128, 1, 1))
    lnb = np.tile(np.asarray(inputs["ln_beta"], np.float32)[None], (128, 1, 1))
    W = np.stack([np.asarray(inputs["W_h"], np.float32),
                  np.asarray(inputs["W_t"], np.float32),
                  np.asarray(inputs["W_r"], np.float32)], axis=0)
    att = np.stack([np.asarray(inputs["att_h"], np.float32),
                    np.asarray(inputs["att_t"], np.float32),
                    np.asarray(inputs["att_r"], np.float32)], axis=0)
    att_rep = np.tile(att.reshape(3, L_LAYERS, 1, HD), (1, 1, 128, 1)).astype(np.float32)
    # Z columns are interleaved as c = 2d+h on device; permute W_o rows to match
    cperm = (np.arange(128) % 2) * 64 + np.arange(128) // 2
    Wo = np.ascontiguousarray(np.asarray(inputs["W_o"], np.float32)[:, cperm, :])
    iota = np.tile(np.repeat(np.arange(128, dtype=np.float32), 2)[None],
                   (128, 1)).astype(BF)
    idf = np.eye(128, dtype=np.float32)
    magic = np.full((128, 1), 0x5f3759df, np.int32)

    common = dict(rel_emb=rel, ln_g=np.ascontiguousarray(lng), ln_b=np.ascontiguousarray(lnb),
                  W_htr=np.ascontiguousarray(W), att_rep=np.ascontiguousarray(att_rep),
                  W_o=np.ascontiguousarray(Wo), iota=iota, ident_f=idf,
                  rsqrt_magic=magic)
    in_maps = []
    for pl in plans:
        sl = np.zeros((M_PER_CORE, DIM), np.float32)
        nv = pl["n_valid"]
        idx = pl["perm"]
        valid = idx < nv
        sl[valid] = ent[pl["node_lo"] + idx[valid]]
        m = dict(common)
        m["ent_slice"] = sl
        m["gidx"] = np.ascontiguousarray(np.stack(pl["gidx"], axis=1))
        m["tidx"] = pl["tidx"]
        m["shidx"] = pl["shidx"]
        m["offs"] = pl["offs"]
        in_maps.append(m)
    return in_maps


_CACHE = {}


def _get_nc(meta, debug, ln_trivial=False):
    key = (meta["PIECE_CHUNKS"], meta["J2BL"], debug, ln_trivial)
    if key not in _CACHE:
        _CACHE[key] = _build_nc(meta, debug=debug, ln_trivial=ln_trivial)
    return _CACHE[key]


def run(inputs, debug=False, trace=False):
    plans, meta = _build_plan(np.asarray(inputs["edge_index"]),
                              np.asarray(inputs["edge_type"]))
    ln_trivial = bool(np.all(np.asarray(inputs["ln_gamma"]) == 1.0)
                      and np.all(np.asarray(inputs["ln_beta"]) == 0.0))
    nc = _get_nc(meta, debug, ln_trivial)
    in_maps = _make_in_maps(inputs, plans)
    res = bass_utils.run_bass_kernel_spmd(nc, in_maps, core_ids=list(range(CORES)),
                                          trace=trace)
    out = np.zeros((N_ENT, DIM), np.float32)
    for c, pl in enumerate(plans):
        nv = pl["n_valid"]
        sl = np.asarray(res.results[c]["out_slice"])
        idx = pl["perm"]
        valid = idx < nv
        out[pl["node_lo"] + idx[valid]] = sl[valid]
    return out, res, plans, meta


def kernel(**inputs) -> np.ndarray:
    out, _, _, _ = run(inputs)
    return out.astype(np.asarray(inputs["entity_embed"]).dtype)

